# revision 6
# baseline (speedup 1.0000x reference)
"""Bicubic grid_sample (transpose-like warp) for Trainium2, 8 NeuronCores.

Strategy: shard output rows across cores (256 rows/core). The warp maps
output (i, j) -> input (y ~ j +- 21, x ~ i +- 21), so each core needs an
x-column slab of the image. On device, repack the slab into a patch table
in DRAM where each 256B unit holds the full 4x4x8ch bicubic patch at
(y0, x0) (fp16). v3: slab ships as f16 (halves slab DMA), gathers merged
to one 8192-idx call per 64-col half-tile (4x fewer calls), weight
products built with fully-contiguous (s,r)-major ops (no strided ACT
copies), and row-group 1's table build is emitted interleaved with
row-group 0's gather/combine so the build streams through the latency
bubbles of the gather phase.
"""
import os, sys, types
sys.path.insert(0, "/opt/trn_rl_repo")
import numpy as np

try:  # register NTFF profile hook so BASS_TRACE=1 can measure HW time
    import antenv
    if "antenv.axon_hooks" not in sys.modules:
        from trn_agent_boot.trn_boot import _ntff_profile_via_ctypes
        _h = _ntff_profile_via_ctypes("/opt/axon/libaxon_pjrt.so")
        _m = types.ModuleType("antenv.axon_hooks")
        _m.get_axon_ntff_profile_hook = lambda: _h
        _m.set_axon_ntff_profile_hook = lambda h: None
        sys.modules["antenv.axon_hooks"] = _m
        antenv.axon_hooks = _m
except Exception:
    pass

import concourse.bass as bass
import concourse.bacc as bacc
import concourse.mybir as mybir
import concourse.tile as tile
from concourse import library_config
from concourse.bass_utils import run_bass_kernel_spmd

F32 = mybir.dt.float32
F16 = mybir.dt.float16
I16 = mybir.dt.int16
I32 = mybir.dt.int32
OP = mybir.AluOpType

N_CORES = 8
H = W = 2048
C = 8
RPC = H // N_CORES          # output rows per core = 256
PAD = 24                    # y halo rows on each side
YS = H + 2 * PAD            # 2096 slab rows
XS = 308                    # slab cols: [I0-24, I0+284)
XT = 176                    # table cols per row-group
XH = 88                     # x-half of the table staging buffer
YT = YS + 16                # table rows incl. pad so in_ap window stays in-bounds
SJW = 512                   # super-tile width (weights/idx granularity)
JW2 = 64                    # half-tile width (gather/combine granularity)
A = -0.75                   # bicubic constant
YB = 124                    # y-block rows for table build
N_YB = (YS + YB - 1) // YB  # 17


def build_nc():
    nc = bacc.Bacc("TRN2", target_bir_lowering=False, debug=False,
                   num_devices=N_CORES, num_swdge_queues=4)
    xs = nc.dram_tensor("xs", [C, YS + 4, XS], F16, kind="ExternalInput")
    gr = nc.dram_tensor("gr", [RPC, W, 2], F32, kind="ExternalInput")
    out = nc.dram_tensor("out", [RPC, W // JW2, C, JW2], F32,
                         kind="ExternalOutput")

    with tile.TileContext(nc) as tc:
        nc.gpsimd.load_library(library_config.mlp)
        import contextlib
        with contextlib.ExitStack() as ctx:
            _build_body(ctx, tc, nc, xs, gr, out)
    nc.compile()
    return nc


def _build_body(ctx, tc, nc, xs, gr, out):
    tabpool = ctx.enter_context(tc.tile_pool(name="tab", bufs=1, space="DRAM"))
    # phase-1 pools
    tpool = ctx.enter_context(tc.tile_pool(name="t", bufs=2))
    tg2pool = ctx.enter_context(tc.tile_pool(name="tg2", bufs=1))
    # phase-2 pools
    gridp = ctx.enter_context(tc.tile_pool(name="grid", bufs=2))
    wrk = ctx.enter_context(tc.tile_pool(name="wrk", bufs=1))
    wpp = ctx.enter_context(tc.tile_pool(name="wpp", bufs=2))
    idxp = ctx.enter_context(tc.tile_pool(name="idx", bufs=2))
    idxs1 = ctx.enter_context(tc.tile_pool(name="idx1", bufs=1))
    gp = ctx.enter_context(tc.tile_pool(name="g", bufs=2))
    lp = ctx.enter_context(tc.tile_pool(name="l", bufs=1))
    outp = ctx.enter_context(tc.tile_pool(name="out", bufs=2))

    tabs = []
    for g in range(2):
        tabg = tabpool.tile([YT * XT, 128], F16, tag=f"tab{g}")
        tabs.append(tabg)

    hwdge = [nc.sync, nc.scalar]
    cnt = {"dma": 0, "cp": 0, "q": 0}

    def eng():
        cnt["dma"] += 1
        return hwdge[cnt["dma"] % 2]

    def ccopy(dst, src):
        cnt["cp"] += 1
        if cnt["cp"] % 2 == 0:
            nc.vector.tensor_copy(dst, src)
        else:
            nc.scalar.copy(dst, src)

    # ---------------- phase 1: repack xs -> table[g], one y-block ----------
    def build_block(g, yb):
        y0 = yb * YB
        rows = min(YB, YS - y0)
        # one DMA per row-shift r covering all 8 channels (slab is f16)
        t4 = tpool.tile([128, 4 * 8 * 179], F16, tag="xsb")
        for r in range(4):
            eng().dma_start(
                bass.AP(t4.tensor, t4.offset + r * 8 * 179,
                        [[t4.ap[0][0], rows], [1, 8 * 179]]),
                bass.AP(xs, (y0 + r) * XS + 128 * g,
                        [[XS, rows], [(YS + 4) * XS, 8], [1, 179]]))
        for h in range(2):
            tg2 = tg2pool.tile([128, XH * 128], F16, tag="tg2")
            for r in range(4):
                # merged interleave+shift:
                # tg2[p, xu*128 + s*32 + r*8 + c] = t[p, c*179 + XH*h + xu + s]
                dst = bass.AP(tg2.tensor, tg2.offset + r * 8,
                              [[tg2.ap[0][0], rows], [32, 4], [128, XH], [1, 8]])
                srcap = bass.AP(t4.tensor, t4.offset + r * 8 * 179 + XH * h,
                                [[t4.ap[0][0], rows], [1, 4], [1, XH], [179, 8]])
                ccopy(dst, srcap)
            # one HWDGE DMA, contiguous 22.5KB per row on both sides
            dsta = bass.AP(tabs[g].tensor,
                           tabs[g].offset + (y0 * XT + h * XH) * 128,
                           [[XT * 128, rows], [1, XH * 128]])
            eng().dma_start(dsta, tg2[:rows, :])

    # ---------------- phase 2: per super-tile weights+idx, gather+combine --
    def cubic(t, tag, outdt, opool):
        # returns w0..w3 tiles [128, SJW] in outdt; all-DVE (no cross-engine
        # handoffs); scratch tags shared between calls (sequential use).
        TS = nc.vector.tensor_scalar
        TT = nc.vector.tensor_tensor
        s0 = wrk.tile([128, SJW], F32, tag="c_s0")
        TS(s0[:], t[:], 1.0, None, op0=OP.add)
        w0f = wrk.tile([128, SJW], F32, tag="c_w0f")
        TS(w0f[:], s0[:], A, -5.0 * A, op0=OP.mult, op1=OP.add)
        TT(w0f[:], w0f[:], s0[:], op=OP.mult)
        TS(w0f[:], w0f[:], 8.0 * A, None, op0=OP.add)
        TT(w0f[:], w0f[:], s0[:], op=OP.mult)
        w0 = opool.tile([128, SJW], outdt, tag=f"w0{tag}")
        TS(w0[:], w0f[:], -4.0 * A, None, op0=OP.add)
        # w1
        w1f = wrk.tile([128, SJW], F32, tag="c_w1f")
        TS(w1f[:], t[:], A + 2.0, -(A + 3.0), op0=OP.mult, op1=OP.add)
        TT(w1f[:], w1f[:], t[:], op=OP.mult)
        TT(w1f[:], w1f[:], t[:], op=OP.mult)
        w1 = opool.tile([128, SJW], outdt, tag=f"w1{tag}")
        TS(w1[:], w1f[:], 1.0, None, op0=OP.add)
        # w2: u = 1 - t
        u = wrk.tile([128, SJW], F32, tag="c_u")
        TS(u[:], t[:], -1.0, 1.0, op0=OP.mult, op1=OP.add)
        w2f = wrk.tile([128, SJW], F32, tag="c_w2f")
        TS(w2f[:], u[:], A + 2.0, -(A + 3.0), op0=OP.mult, op1=OP.add)
        TT(w2f[:], w2f[:], u[:], op=OP.mult)
        TT(w2f[:], w2f[:], u[:], op=OP.mult)
        w2 = opool.tile([128, SJW], outdt, tag=f"w2{tag}")
        TS(w2[:], w2f[:], 1.0, None, op0=OP.add)
        # w3 = 1 - w0 - w1 - w2 (in f32 then cast)
        w3f = wrk.tile([128, SJW], F32, tag="c_w3f")
        TT(w3f[:], w0[:], w1[:], op=OP.add)
        TT(w3f[:], w3f[:], w2[:], op=OP.add)
        w3 = opool.tile([128, SJW], outdt, tag=f"w3{tag}")
        TS(w3[:], w3f[:], -1.0, 1.0, op0=OP.mult, op1=OP.add)
        return [w0, w1, w2, w3]

    def floorpair(v, tag):
        # vi/co scratch shared between calls; vf/fr persist per-dir
        vi = wrk.tile([128, SJW], I32, tag="f_vi")
        nc.vector.tensor_copy(vi[:], v[:])
        vf = wrk.tile([128, SJW], F32, tag=f"vf{tag}")
        nc.vector.tensor_copy(vf[:], vi[:])
        co = wrk.tile([128, SJW], F32, tag="f_co")
        nc.vector.tensor_tensor(co[:], vf[:], v[:], op=OP.is_gt)
        nc.vector.tensor_tensor(vf[:], vf[:], co[:], op=OP.subtract)
        fr = wrk.tile([128, SJW], F32, tag=f"fr{tag}")
        nc.vector.tensor_tensor(fr[:], v[:], vf[:], op=OP.subtract)
        return vf, fr

    def super_tile(g, s4):
        """Weights + wrapped idx for 512 output cols of row-group g.
        Returns (wp2, Cw): wp2[i, (s*4+r)*SJW + j] = wx_s(i,j)*wy_r(i,j)."""
        IG = g * 128
        jb4 = s4 * SJW
        gt = gridp.tile([128, SJW * 2], F32, tag="gt")
        eng().dma_start(
            gt[:],
            bass.AP(gr, IG * W * 2 + jb4 * 2, [[W * 2, 128], [1, SJW * 2]]))
        gx = bass.AP(gt.tensor, gt.offset, [gt.ap[0], [2, SJW]])
        gy = bass.AP(gt.tensor, gt.offset + 1, [gt.ap[0], [2, SJW]])

        lx = wrk.tile([128, SJW], F32, tag="lx")
        ly = wrk.tile([128, SJW], F32, tag="ly")
        nc.vector.tensor_scalar(lx[:], gx, 1024.0, 1047.5 - IG,
                                op0=OP.mult, op1=OP.add)
        nc.vector.tensor_scalar(ly[:], gy, 1024.0, 1046.5 - jb4,
                                op0=OP.mult, op1=OP.add)
        fx, tx = floorpair(lx, "x")
        fy, ty = floorpair(ly, "y")

        # idxf = fy*XT + fx - 1 (f32, exact)
        idxf = wrk.tile([128, SJW], F32, tag="idxf")
        nc.vector.scalar_tensor_tensor(idxf[:], fy[:], float(XT), fx[:],
                                       op0=OP.mult, op1=OP.add)
        # per-sub-tile rebase to the 186-row gather window, cast to i16
        idx16 = idxs1.tile([128, SJW], I16, tag="idx16")
        for t in range(SJW // 128):
            nc.vector.tensor_scalar(
                bass.AP(idx16.tensor, idx16.offset + t * 128,
                        [[idx16.ap[0][0], 128], [1, 128]]),
                bass.AP(idxf.tensor, idxf.offset + t * 128,
                        [[idxf.ap[0][0], 128], [1, 128]]),
                -1.0 - t * 128.0 * XT, None, op0=OP.add)

        # fold [128, SJW] -> wrapped [16, 8*SJW]: D[p, k*SJW + j] = idx16[16k+p, j]
        D = idxs1.tile([128, 8 * SJW], I16, tag="D")
        for k in range(8):
            src = bass.AP(idx16.tensor,
                          idx16.offset + 16 * k * idx16.ap[0][0],
                          [[idx16.ap[0][0], 16], [1, SJW]])
            dst = bass.AP(D.tensor, D.offset + k * SJW,
                          [[D.ap[0][0], 16], [1, SJW]])
            eng().dma_start(dst, src)
        # interleave: Cw[p, 8j+k] = D[p, k*SJW + j]  (one strided copy)
        Cw = idxp.tile([128, 8 * SJW], I16, tag="Cw")
        nc.vector.tensor_copy(
            bass.AP(Cw.tensor, Cw.offset, [[Cw.ap[0][0], 16], [8, SJW], [1, 8]]),
            bass.AP(D.tensor, D.offset, [[D.ap[0][0], 16], [1, SJW], [SJW, 8]]))
        # replicate to all 8 gpsimd cores (doubling: 16 -> 32 -> 64 -> 128)
        for rep in (16, 32, 64):
            src = bass.AP(Cw.tensor, Cw.offset, [[Cw.ap[0][0], rep], [1, 8 * SJW]])
            dst = bass.AP(Cw.tensor, Cw.offset + rep * Cw.ap[0][0],
                          [[Cw.ap[0][0], rep], [1, 8 * SJW]])
            eng().dma_start(dst, src)

        wx = cubic(tx, "x", F16, wrk)
        wy = cubic(ty, "y", F16, wrk)
        # wp2[i, (s*4+r)*SJW + j]: 16 fully-contiguous DVE multiplies
        wp2 = wpp.tile([128, 16 * SJW], F16, tag="wp2")
        for s in range(4):
            for r in range(4):
                dst = bass.AP(wp2.tensor, wp2.offset + (s * 4 + r) * SJW,
                              [wp2.ap[0], [1, SJW]])
                nc.vector.tensor_tensor(dst, wx[s][:], wy[r][:], op=OP.mult)
        return wp2, Cw

    def half_tile(g, s4, t, h, wp2, Cw):
        IG = g * 128
        jb = s4 * SJW + t * 128 + h * JW2
        ybase = s4 * SJW + t * 128
        joff = t * 128 + h * JW2

        # one 8192-idx gather: G[i, jl*128 + e], e = s*32 + r*8 + c
        G = gp.tile([128, JW2 * 128], F16, tag="G")
        in_ap = bass.AP(tabs[g].tensor,
                        tabs[g].offset + ybase * XT * 128,
                        [[128, 186 * XT], [1, 128]])
        NSUB = 8192
        idxsl = bass.AP(Cw.tensor, Cw.offset + joff * 8,
                        [[Cw.ap[0][0], 128], [1, NSUB // 16]])
        q = cnt["q"] % 4
        cnt["q"] += 1
        nc.gpsimd.dma_gather(
            out_ap=bass.AP(G.tensor, G.offset,
                           [[G.ap[0][0], 128], [128, JW2], [1, 128]]),
            in_ap=in_ap,
            idxs_ap=idxsl,
            num_idxs=NSUB,
            num_idxs_reg=NSUB,
            elem_size=128,
            elem_step=128,
            single_packet=False,
            queue_num=q,
        )

        # combine: G[i, jl*128 + s*32 + r*8 + c] *= wp2[i, (s*4+r)*SJW + jb+jl]
        # (bcast over c) in two halves
        for m in range(2):
            src1 = bass.AP(wp2.tensor, wp2.offset + joff + m * 32,
                           [wp2.ap[0], [1, JW2 // 2], [4 * SJW, 4], [SJW, 4],
                            [0, 8]])
            src0 = bass.AP(G.tensor, G.offset + m * 32 * 128,
                           [G.ap[0], [128, JW2 // 2], [32, 4], [8, 4], [1, 8]])
            nc.vector.tensor_tensor(src0, src0, src1, op=OP.mult)

        def halve(buf, stride, n, tag, npx=JW2):
            o = lp.tile([128, npx * stride * (n // 2)], F16, tag=tag)
            i0 = bass.AP(buf.tensor, buf.offset,
                         [buf.ap[0], [stride * n, npx], [stride * 2, n // 2], [1, stride]])
            i1 = bass.AP(buf.tensor, buf.offset + stride,
                         [buf.ap[0], [stride * n, npx], [stride * 2, n // 2], [1, stride]])
            od = bass.AP(o.tensor, o.offset,
                         [o.ap[0], [stride * (n // 2), npx], [stride, n // 2], [1, stride]])
            nc.vector.tensor_tensor(od, i0, i1, op=OP.add)
            return o

        L1 = halve(G, 32, 4, "L1")
        L2 = halve(L1, 32, 2, "L2")
        L3 = halve(L2, 8, 4, "L3")
        of = outp.tile([128, 8 * JW2], F32, tag="of")
        i0 = bass.AP(L3.tensor, L3.offset, [L3.ap[0], [1, 8], [16, JW2]])
        i1 = bass.AP(L3.tensor, L3.offset + 8, [L3.ap[0], [1, 8], [16, JW2]])
        od = bass.AP(of.tensor, of.offset, [of.ap[0], [JW2, 8], [1, JW2]])
        nc.vector.tensor_tensor(od, i0, i1, op=OP.add)

        # blocked layout [RPC, W//64, C, 64]: 2KB contiguous per partition
        dsto = bass.AP(out, (IG * (W // JW2) + jb // JW2) * C * JW2,
                       [[(W // JW2) * C * JW2, 128], [1, C * JW2]])
        eng().dma_start(dsto, of[:])

    # ---------------- emission schedule --------------------------------
    # Build tab[0] first (minimum to start g=0 gathers); tab[1]'s build is
    # interleaved into g=0's half-tile stream so its streaming DMAs fill
    # the gather latency bubbles. Supers are software-pipelined one ahead.
    for yb in range(N_YB):
        build_block(0, yb)

    run_order = [(0, 0), (0, 1), (0, 2), (0, 3), (1, 0), (1, 1), (1, 2), (1, 3)]
    build1 = list(range(N_YB))  # tab[1] blocks pending emission
    sups = {run_order[0]: super_tile(*run_order[0])}
    for i, (g, s4) in enumerate(run_order):
        if i + 1 < len(run_order):
            gn, sn = run_order[i + 1]
            sups[(gn, sn)] = super_tile(gn, sn)
        for t in range(4):
            for h in range(2):
                half_tile(g, s4, t, h, *sups[(g, s4)])
                # sprinkle tab[1] build under g=0's gather stream
                if g == 0 and build1:
                    build_block(1, build1.pop(0))
        # all tab[1] blocks must be emitted before the first g=1 half-tile
        if g == 0 and s4 == 3:
            while build1:
                build_block(1, build1.pop(0))
        del sups[(g, s4)]


_NC_CACHE = None


def kernel(x: np.ndarray, grid: np.ndarray) -> np.ndarray:
    global _NC_CACHE
    if _NC_CACHE is None:
        _NC_CACHE = build_nc()
    nc = _NC_CACHE

    x0 = np.ascontiguousarray(x[0], dtype=np.float32)        # [C, H, W]
    g0 = np.ascontiguousarray(grid[0], dtype=np.float32)     # [H, W, 2]

    in_maps = []
    for k in range(N_CORES):
        I0 = k * RPC
        xsl = np.zeros((C, YS + 4, XS), dtype=np.float16)
        c0 = I0 - PAD
        lo, hi = max(0, c0), min(W, c0 + XS)
        xsl[:, PAD:PAD + H, lo - c0:hi - c0] = x0[:, :, lo:hi].astype(np.float16)
        grc = np.ascontiguousarray(g0[I0:I0 + RPC]).copy()
        grc[..., 0] -= I0 / 1024.0   # fold per-core x-base into gx
        in_maps.append({"xs": xsl, "gr": grc})

    res = run_bass_kernel_spmd(nc, in_maps, core_ids=list(range(N_CORES)),
                               trace=False)
    global _LAST_EXEC_NS, _LAST_RES
    _LAST_EXEC_NS = res.exec_time_ns
    _LAST_RES = res
    out = np.empty((1, C, H, W), dtype=np.float32)
    for k in range(N_CORES):
        blk = res.results[k]["out"]          # [RPC, W//64, C, 64]
        out[0, :, k * RPC:(k + 1) * RPC, :] = (
            blk.transpose(2, 0, 1, 3).reshape(C, RPC, W))
    return out


# revision 8
# speedup vs baseline: 1.2416x; 1.2416x over previous
"""Bicubic grid_sample (transpose-like warp) for Trainium2, 8 NeuronCores.

Strategy: shard output rows across cores (256 rows/core). The warp maps
output (i, j) -> input (y ~ j +- 21, x ~ i +- 21), so each core needs an
x-column slab of the image. On device, repack the slab into a patch table
in DRAM where each 256B unit holds the full 4x4x8ch bicubic patch at
(y0, x0) (fp16). v3: slab ships as f16 (halves slab DMA), gathers merged
to one 8192-idx call per 64-col half-tile (4x fewer calls), weight
products built with fully-contiguous (s,r)-major ops (no strided ACT
copies), and row-group 1's table build is emitted interleaved with
row-group 0's gather/combine so the build streams through the latency
bubbles of the gather phase.
"""
import os, sys, types
sys.path.insert(0, "/opt/trn_rl_repo")
import numpy as np

try:  # register NTFF profile hook so BASS_TRACE=1 can measure HW time
    import antenv
    if "antenv.axon_hooks" not in sys.modules:
        from trn_agent_boot.trn_boot import _ntff_profile_via_ctypes
        _h = _ntff_profile_via_ctypes("/opt/axon/libaxon_pjrt.so")
        _m = types.ModuleType("antenv.axon_hooks")
        _m.get_axon_ntff_profile_hook = lambda: _h
        _m.set_axon_ntff_profile_hook = lambda h: None
        sys.modules["antenv.axon_hooks"] = _m
        antenv.axon_hooks = _m
except Exception:
    pass

import concourse.bass as bass
import concourse.bacc as bacc
import concourse.mybir as mybir
import concourse.tile as tile
from concourse import library_config
from concourse.bass_utils import run_bass_kernel_spmd

F32 = mybir.dt.float32
F16 = mybir.dt.float16
I16 = mybir.dt.int16
I32 = mybir.dt.int32
OP = mybir.AluOpType

N_CORES = 8
H = W = 2048
C = 8
RPC = H // N_CORES          # output rows per core = 256
PAD = 24                    # y halo rows on each side
YS = H + 2 * PAD            # 2096 slab rows
XS = 308                    # slab cols: [I0-24, I0+284)
XT = 176                    # table cols per row-group
XH = 88                     # x-half of the table staging buffer
YT = YS + 16                # table rows incl. pad so in_ap window stays in-bounds
SJW = 512                   # super-tile width (weights/idx granularity)
JW2 = 64                    # half-tile width (gather/combine granularity)
A = -0.75                   # bicubic constant
YB = 124                    # y-block rows for table build
N_YB = (YS + YB - 1) // YB  # 17


def build_nc():
    nc = bacc.Bacc("TRN2", target_bir_lowering=False, debug=False,
                   num_devices=N_CORES, num_swdge_queues=4)
    xs = nc.dram_tensor("xs", [C, YS + 4, XS], F16, kind="ExternalInput")
    gr = nc.dram_tensor("gr", [RPC, W, 2], F32, kind="ExternalInput")
    out = nc.dram_tensor("out", [RPC, W // JW2, C, JW2], F32,
                         kind="ExternalOutput")

    with tile.TileContext(nc) as tc:
        nc.gpsimd.load_library(library_config.mlp)
        import contextlib
        with contextlib.ExitStack() as ctx:
            _build_body(ctx, tc, nc, xs, gr, out)
    nc.compile()
    return nc


def _build_body(ctx, tc, nc, xs, gr, out):
    tabpool = ctx.enter_context(tc.tile_pool(name="tab", bufs=1, space="DRAM"))
    # phase-1 pools
    tpool = ctx.enter_context(tc.tile_pool(name="t", bufs=2))
    tg2pool = ctx.enter_context(tc.tile_pool(name="tg2", bufs=1))
    # phase-2 pools
    gridp = ctx.enter_context(tc.tile_pool(name="grid", bufs=2))
    wrk = ctx.enter_context(tc.tile_pool(name="wrk", bufs=1))
    wpp = ctx.enter_context(tc.tile_pool(name="wpp", bufs=2))
    idxp = ctx.enter_context(tc.tile_pool(name="idx", bufs=2))
    idxs1 = ctx.enter_context(tc.tile_pool(name="idx1", bufs=1))
    gp = ctx.enter_context(tc.tile_pool(name="g", bufs=2))
    lp = ctx.enter_context(tc.tile_pool(name="l", bufs=1))
    outp = ctx.enter_context(tc.tile_pool(name="out", bufs=2))

    tabs = []
    for g in range(2):
        tabg = tabpool.tile([YT * XT, 128], F16, tag=f"tab{g}")
        tabs.append(tabg)

    hwdge = [nc.sync, nc.scalar]
    cnt = {"dma": 0, "cp": 0, "q": 0}

    def eng():
        cnt["dma"] += 1
        return hwdge[cnt["dma"] % 2]

    def ccopy(dst, src):
        cnt["cp"] += 1
        if cnt["cp"] % 2 == 0:
            nc.vector.tensor_copy(dst, src)
        else:
            nc.scalar.copy(dst, src)

    # ---------------- phase 1: repack xs -> table[g], one y-block ----------
    def build_block(g, yb):
        y0 = yb * YB
        rows = min(YB, YS - y0)
        # one DMA per row-shift r covering all 8 channels (slab is f16)
        t4 = tpool.tile([128, 4 * 8 * 179], F16, tag="xsb")
        for r in range(4):
            eng().dma_start(
                bass.AP(t4.tensor, t4.offset + r * 8 * 179,
                        [[t4.ap[0][0], rows], [1, 8 * 179]]),
                bass.AP(xs, (y0 + r) * XS + 128 * g,
                        [[XS, rows], [(YS + 4) * XS, 8], [1, 179]]))
        for h in range(2):
            tg2 = tg2pool.tile([128, XH * 128], F16, tag="tg2")
            for r in range(4):
                # merged interleave+shift:
                # tg2[p, xu*128 + s*32 + r*8 + c] = t[p, c*179 + XH*h + xu + s]
                dst = bass.AP(tg2.tensor, tg2.offset + r * 8,
                              [[tg2.ap[0][0], rows], [32, 4], [128, XH], [1, 8]])
                srcap = bass.AP(t4.tensor, t4.offset + r * 8 * 179 + XH * h,
                                [[t4.ap[0][0], rows], [1, 4], [1, XH], [179, 8]])
                ccopy(dst, srcap)
            # one HWDGE DMA, contiguous 22.5KB per row on both sides
            dsta = bass.AP(tabs[g].tensor,
                           tabs[g].offset + (y0 * XT + h * XH) * 128,
                           [[XT * 128, rows], [1, XH * 128]])
            eng().dma_start(dsta, tg2[:rows, :])

    # ---------------- phase 2: per super-tile weights+idx, gather+combine --
    def cubic(t, tag, outdt, opool):
        # returns w0..w3 tiles [128, SJW] in outdt; all-DVE (no cross-engine
        # handoffs); scratch tags shared between calls (sequential use).
        TS = nc.vector.tensor_scalar
        TT = nc.vector.tensor_tensor
        s0 = wrk.tile([128, SJW], F32, tag="c_s0")
        TS(s0[:], t[:], 1.0, None, op0=OP.add)
        w0f = wrk.tile([128, SJW], F32, tag="c_w0f")
        TS(w0f[:], s0[:], A, -5.0 * A, op0=OP.mult, op1=OP.add)
        TT(w0f[:], w0f[:], s0[:], op=OP.mult)
        TS(w0f[:], w0f[:], 8.0 * A, None, op0=OP.add)
        TT(w0f[:], w0f[:], s0[:], op=OP.mult)
        w0 = opool.tile([128, SJW], outdt, tag=f"w0{tag}")
        TS(w0[:], w0f[:], -4.0 * A, None, op0=OP.add)
        # w1
        w1f = wrk.tile([128, SJW], F32, tag="c_w1f")
        TS(w1f[:], t[:], A + 2.0, -(A + 3.0), op0=OP.mult, op1=OP.add)
        TT(w1f[:], w1f[:], t[:], op=OP.mult)
        TT(w1f[:], w1f[:], t[:], op=OP.mult)
        w1 = opool.tile([128, SJW], outdt, tag=f"w1{tag}")
        TS(w1[:], w1f[:], 1.0, None, op0=OP.add)
        # w2: u = 1 - t
        u = wrk.tile([128, SJW], F32, tag="c_u")
        TS(u[:], t[:], -1.0, 1.0, op0=OP.mult, op1=OP.add)
        w2f = wrk.tile([128, SJW], F32, tag="c_w2f")
        TS(w2f[:], u[:], A + 2.0, -(A + 3.0), op0=OP.mult, op1=OP.add)
        TT(w2f[:], w2f[:], u[:], op=OP.mult)
        TT(w2f[:], w2f[:], u[:], op=OP.mult)
        w2 = opool.tile([128, SJW], outdt, tag=f"w2{tag}")
        TS(w2[:], w2f[:], 1.0, None, op0=OP.add)
        # w3 = 1 - w0 - w1 - w2 (in f32 then cast)
        w3f = wrk.tile([128, SJW], F32, tag="c_w3f")
        TT(w3f[:], w0[:], w1[:], op=OP.add)
        TT(w3f[:], w3f[:], w2[:], op=OP.add)
        w3 = opool.tile([128, SJW], outdt, tag=f"w3{tag}")
        TS(w3[:], w3f[:], -1.0, 1.0, op0=OP.mult, op1=OP.add)
        return [w0, w1, w2, w3]

    def floorpair(v, tag):
        # vi/co scratch shared between calls; vf/fr persist per-dir
        vi = wrk.tile([128, SJW], I32, tag="f_vi")
        nc.vector.tensor_copy(vi[:], v[:])
        vf = wrk.tile([128, SJW], F32, tag=f"vf{tag}")
        nc.vector.tensor_copy(vf[:], vi[:])
        co = wrk.tile([128, SJW], F32, tag="f_co")
        nc.vector.tensor_tensor(co[:], vf[:], v[:], op=OP.is_gt)
        nc.vector.tensor_tensor(vf[:], vf[:], co[:], op=OP.subtract)
        fr = wrk.tile([128, SJW], F32, tag=f"fr{tag}")
        nc.vector.tensor_tensor(fr[:], v[:], vf[:], op=OP.subtract)
        return vf, fr

    def super_tile(g, s4):
        """Weights + wrapped idx for 512 output cols of row-group g.
        Returns (wp2, Cw): wp2[i, (s*4+r)*SJW + j] = wx_s(i,j)*wy_r(i,j)."""
        IG = g * 128
        jb4 = s4 * SJW
        gt = gridp.tile([128, SJW * 2], F32, tag="gt")
        eng().dma_start(
            gt[:],
            bass.AP(gr, IG * W * 2 + jb4 * 2, [[W * 2, 128], [1, SJW * 2]]))
        gx = bass.AP(gt.tensor, gt.offset, [gt.ap[0], [2, SJW]])
        gy = bass.AP(gt.tensor, gt.offset + 1, [gt.ap[0], [2, SJW]])

        lx = wrk.tile([128, SJW], F32, tag="lx")
        ly = wrk.tile([128, SJW], F32, tag="ly")
        nc.vector.tensor_scalar(lx[:], gx, 1024.0, 1047.5 - IG,
                                op0=OP.mult, op1=OP.add)
        nc.vector.tensor_scalar(ly[:], gy, 1024.0, 1046.5 - jb4,
                                op0=OP.mult, op1=OP.add)
        fx, tx = floorpair(lx, "x")
        fy, ty = floorpair(ly, "y")

        # idxf = fy*XT + fx - 1 (f32, exact)
        idxf = wrk.tile([128, SJW], F32, tag="idxf")
        nc.vector.scalar_tensor_tensor(idxf[:], fy[:], float(XT), fx[:],
                                       op0=OP.mult, op1=OP.add)
        # per-sub-tile rebase to the 186-row gather window, cast to i16
        idx16 = idxs1.tile([128, SJW], I16, tag="idx16")
        for t in range(SJW // 128):
            nc.vector.tensor_scalar(
                bass.AP(idx16.tensor, idx16.offset + t * 128,
                        [[idx16.ap[0][0], 128], [1, 128]]),
                bass.AP(idxf.tensor, idxf.offset + t * 128,
                        [[idxf.ap[0][0], 128], [1, 128]]),
                -1.0 - t * 128.0 * XT, None, op0=OP.add)

        # fold [128, SJW] -> wrapped [16, 8*SJW]: D[p, k*SJW + j] = idx16[16k+p, j]
        D = idxs1.tile([128, 8 * SJW], I16, tag="D")
        for k in range(8):
            src = bass.AP(idx16.tensor,
                          idx16.offset + 16 * k * idx16.ap[0][0],
                          [[idx16.ap[0][0], 16], [1, SJW]])
            dst = bass.AP(D.tensor, D.offset + k * SJW,
                          [[D.ap[0][0], 16], [1, SJW]])
            eng().dma_start(dst, src)
        # interleave: Cw[p, 8j+k] = D[p, k*SJW + j]  (one strided copy)
        Cw = idxp.tile([128, 8 * SJW], I16, tag="Cw")
        nc.vector.tensor_copy(
            bass.AP(Cw.tensor, Cw.offset, [[Cw.ap[0][0], 16], [8, SJW], [1, 8]]),
            bass.AP(D.tensor, D.offset, [[D.ap[0][0], 16], [1, SJW], [SJW, 8]]))
        # replicate to all 8 gpsimd cores (doubling: 16 -> 32 -> 64 -> 128)
        for rep in (16, 32, 64):
            src = bass.AP(Cw.tensor, Cw.offset, [[Cw.ap[0][0], rep], [1, 8 * SJW]])
            dst = bass.AP(Cw.tensor, Cw.offset + rep * Cw.ap[0][0],
                          [[Cw.ap[0][0], rep], [1, 8 * SJW]])
            eng().dma_start(dst, src)

        wx = cubic(tx, "x", F16, wrk)
        wy = cubic(ty, "y", F16, wrk)
        # wp2[i, (s*4+r)*SJW + j]: 16 fully-contiguous DVE multiplies
        wp2 = wpp.tile([128, 16 * SJW], F16, tag="wp2")
        for s in range(4):
            for r in range(4):
                dst = bass.AP(wp2.tensor, wp2.offset + (s * 4 + r) * SJW,
                              [wp2.ap[0], [1, SJW]])
                nc.vector.tensor_tensor(dst, wx[s][:], wy[r][:], op=OP.mult)
        return wp2, Cw

    def half_tile(g, s4, t, h, wp2, Cw):
        IG = g * 128
        jb = s4 * SJW + t * 128 + h * JW2
        ybase = s4 * SJW + t * 128
        joff = t * 128 + h * JW2

        # four 2048-idx gathers (ring-resident, 4-queue pipelined):
        # G[i, jl*128 + e], e = s*32 + r*8 + c
        G = gp.tile([128, JW2 * 128], F16, tag="G")
        in_ap = bass.AP(tabs[g].tensor,
                        tabs[g].offset + ybase * XT * 128,
                        [[128, 186 * XT], [1, 128]])
        NSUB = 2048
        for m in range(4):
            idxsl = bass.AP(Cw.tensor, Cw.offset + joff * 8 + m * (NSUB // 16),
                            [[Cw.ap[0][0], 128], [1, NSUB // 16]])
            q = cnt["q"] % 4
            cnt["q"] += 1
            nc.gpsimd.dma_gather(
                out_ap=bass.AP(G.tensor, G.offset + m * 16 * 128,
                               [[G.ap[0][0], 128], [128, 16], [1, 128]]),
                in_ap=in_ap,
                idxs_ap=idxsl,
                num_idxs=NSUB,
                num_idxs_reg=NSUB,
                elem_size=128,
                elem_step=128,
                single_packet=False,
                queue_num=q,
            )

        # combine: G[i, jl*128 + s*32 + r*8 + c] *= wp2[i, (s*4+r)*SJW + jb+jl]
        # (bcast over c) in two halves
        for m in range(2):
            src1 = bass.AP(wp2.tensor, wp2.offset + joff + m * 32,
                           [wp2.ap[0], [1, JW2 // 2], [4 * SJW, 4], [SJW, 4],
                            [0, 8]])
            src0 = bass.AP(G.tensor, G.offset + m * 32 * 128,
                           [G.ap[0], [128, JW2 // 2], [32, 4], [8, 4], [1, 8]])
            nc.vector.tensor_tensor(src0, src0, src1, op=OP.mult)

        def halve(buf, stride, n, tag, npx=JW2):
            o = lp.tile([128, npx * stride * (n // 2)], F16, tag=tag)
            i0 = bass.AP(buf.tensor, buf.offset,
                         [buf.ap[0], [stride * n, npx], [stride * 2, n // 2], [1, stride]])
            i1 = bass.AP(buf.tensor, buf.offset + stride,
                         [buf.ap[0], [stride * n, npx], [stride * 2, n // 2], [1, stride]])
            od = bass.AP(o.tensor, o.offset,
                         [o.ap[0], [stride * (n // 2), npx], [stride, n // 2], [1, stride]])
            nc.vector.tensor_tensor(od, i0, i1, op=OP.add)
            return o

        L1 = halve(G, 32, 4, "L1")
        L2 = halve(L1, 32, 2, "L2")
        L3 = halve(L2, 8, 4, "L3")
        of = outp.tile([128, 8 * JW2], F32, tag="of")
        i0 = bass.AP(L3.tensor, L3.offset, [L3.ap[0], [1, 8], [16, JW2]])
        i1 = bass.AP(L3.tensor, L3.offset + 8, [L3.ap[0], [1, 8], [16, JW2]])
        od = bass.AP(of.tensor, of.offset, [of.ap[0], [JW2, 8], [1, JW2]])
        nc.vector.tensor_tensor(od, i0, i1, op=OP.add)

        # blocked layout [RPC, W//64, C, 64]: 2KB contiguous per partition
        dsto = bass.AP(out, (IG * (W // JW2) + jb // JW2) * C * JW2,
                       [[(W // JW2) * C * JW2, 128], [1, C * JW2]])
        eng().dma_start(dsto, of[:])

    # ---------------- emission schedule --------------------------------
    # Serial phases: build both tables first (gather desc-gen on GpSimd
    # thrashes the SBUF ports DVE needs, so overlapping the build with the
    # gather phase slows the interleave copies ~5x). Supers are
    # software-pipelined one ahead so weights compute during gathers.
    for yb in range(N_YB):
        build_block(0, yb)
        build_block(1, yb)

    run_order = [(0, 0), (1, 0), (0, 1), (1, 1), (0, 2), (1, 2), (0, 3), (1, 3)]
    sups = {run_order[0]: super_tile(*run_order[0])}
    for i, (g, s4) in enumerate(run_order):
        if i + 1 < len(run_order):
            gn, sn = run_order[i + 1]
            sups[(gn, sn)] = super_tile(gn, sn)
        for t in range(4):
            for h in range(2):
                half_tile(g, s4, t, h, *sups[(g, s4)])
        del sups[(g, s4)]


_NC_CACHE = None


def kernel(x: np.ndarray, grid: np.ndarray) -> np.ndarray:
    global _NC_CACHE
    if _NC_CACHE is None:
        _NC_CACHE = build_nc()
    nc = _NC_CACHE

    x0 = np.ascontiguousarray(x[0], dtype=np.float32)        # [C, H, W]
    g0 = np.ascontiguousarray(grid[0], dtype=np.float32)     # [H, W, 2]

    in_maps = []
    for k in range(N_CORES):
        I0 = k * RPC
        xsl = np.zeros((C, YS + 4, XS), dtype=np.float16)
        c0 = I0 - PAD
        lo, hi = max(0, c0), min(W, c0 + XS)
        xsl[:, PAD:PAD + H, lo - c0:hi - c0] = x0[:, :, lo:hi].astype(np.float16)
        grc = np.ascontiguousarray(g0[I0:I0 + RPC]).copy()
        grc[..., 0] -= I0 / 1024.0   # fold per-core x-base into gx
        in_maps.append({"xs": xsl, "gr": grc})

    res = run_bass_kernel_spmd(nc, in_maps, core_ids=list(range(N_CORES)),
                               trace=False)
    global _LAST_EXEC_NS, _LAST_RES
    _LAST_EXEC_NS = res.exec_time_ns
    _LAST_RES = res
    out = np.empty((1, C, H, W), dtype=np.float32)
    for k in range(N_CORES):
        blk = res.results[k]["out"]          # [RPC, W//64, C, 64]
        out[0, :, k * RPC:(k + 1) * RPC, :] = (
            blk.transpose(2, 0, 1, 3).reshape(C, RPC, W))
    return out


# revision 10
# speedup vs baseline: 1.2469x; 1.0043x over previous
"""Bicubic grid_sample (transpose-like warp) for Trainium2, 8 NeuronCores.

Strategy: shard output rows across cores (256 rows/core). The warp maps
output (i, j) -> input (y ~ j +- 21, x ~ i +- 21), so each core needs an
x-column slab of the image. On device, repack the slab into a patch table
in DRAM where each 256B unit holds the full 4x4x8ch bicubic patch at
(y0, x0) (fp16). v3: slab ships as f16 (halves slab DMA), gathers merged
to one 8192-idx call per 64-col half-tile (4x fewer calls), weight
products built with fully-contiguous (s,r)-major ops (no strided ACT
copies), and row-group 1's table build is emitted interleaved with
row-group 0's gather/combine so the build streams through the latency
bubbles of the gather phase.
"""
import os, sys, types
sys.path.insert(0, "/opt/trn_rl_repo")
import numpy as np

try:  # register NTFF profile hook so BASS_TRACE=1 can measure HW time
    import antenv
    if "antenv.axon_hooks" not in sys.modules:
        from trn_agent_boot.trn_boot import _ntff_profile_via_ctypes
        _h = _ntff_profile_via_ctypes("/opt/axon/libaxon_pjrt.so")
        _m = types.ModuleType("antenv.axon_hooks")
        _m.get_axon_ntff_profile_hook = lambda: _h
        _m.set_axon_ntff_profile_hook = lambda h: None
        sys.modules["antenv.axon_hooks"] = _m
        antenv.axon_hooks = _m
except Exception:
    pass

import concourse.bass as bass
import concourse.bacc as bacc
import concourse.mybir as mybir
import concourse.tile as tile
from concourse import library_config
from concourse.bass_utils import run_bass_kernel_spmd

F32 = mybir.dt.float32
F16 = mybir.dt.float16
I16 = mybir.dt.int16
I32 = mybir.dt.int32
OP = mybir.AluOpType

N_CORES = 8
H = W = 2048
C = 8
RPC = H // N_CORES          # output rows per core = 256
PAD = 24                    # y halo rows on each side
YS = H + 2 * PAD            # 2096 slab rows
XS = 308                    # slab cols: [I0-24, I0+284)
XT = 176                    # table cols per row-group
XH = 88                     # x-half of the table staging buffer
YT = YS + 16                # table rows incl. pad so in_ap window stays in-bounds
SJW = 512                   # super-tile width (weights/idx granularity)
JW2 = 64                    # half-tile width (gather/combine granularity)
A = -0.75                   # bicubic constant
YB = 124                    # y-block rows for table build
N_YB = (YS + YB - 1) // YB  # 17


def build_nc():
    nc = bacc.Bacc("TRN2", target_bir_lowering=False, debug=False,
                   num_devices=N_CORES, num_swdge_queues=4)
    xs = nc.dram_tensor("xs", [C, YS + 4, XS], F16, kind="ExternalInput")
    gr = nc.dram_tensor("gr", [RPC, W, 2], F32, kind="ExternalInput")
    out = nc.dram_tensor("out", [RPC, W // JW2, C, JW2], F32,
                         kind="ExternalOutput")

    with tile.TileContext(nc) as tc:
        nc.gpsimd.load_library(library_config.mlp)
        import contextlib
        with contextlib.ExitStack() as ctx:
            _build_body(ctx, tc, nc, xs, gr, out)
    nc.compile()
    return nc


def _build_body(ctx, tc, nc, xs, gr, out):
    tabpool = ctx.enter_context(tc.tile_pool(name="tab", bufs=1, space="DRAM"))
    # phase-1 pools
    tpool = ctx.enter_context(tc.tile_pool(name="t", bufs=2))
    tg2pool = ctx.enter_context(tc.tile_pool(name="tg2", bufs=1))
    # phase-2 pools
    gridp = ctx.enter_context(tc.tile_pool(name="grid", bufs=2))
    wrk = ctx.enter_context(tc.tile_pool(name="wrk", bufs=1))
    wpp = ctx.enter_context(tc.tile_pool(name="wpp", bufs=2))
    idxp = ctx.enter_context(tc.tile_pool(name="idx", bufs=2))
    idxs1 = ctx.enter_context(tc.tile_pool(name="idx1", bufs=1))
    gp = ctx.enter_context(tc.tile_pool(name="g", bufs=2))
    lp = ctx.enter_context(tc.tile_pool(name="l", bufs=1))
    outp = ctx.enter_context(tc.tile_pool(name="out", bufs=2))

    tabs = []
    for g in range(2):
        tabg = tabpool.tile([YT * XT, 128], F16, tag=f"tab{g}")
        tabs.append(tabg)

    hwdge = [nc.sync, nc.scalar]
    cnt = {"dma": 0, "cp": 0, "q": 0}

    def eng():
        # phase-1 bulk traffic: alternate both HWDGE engines
        cnt["dma"] += 1
        return hwdge[cnt["dma"] % 2]

    def eng_prep():
        # phase-2 latency-critical prep (grid, idx folds, replication):
        # keep off the output-write queue to avoid head-of-line blocking
        return nc.sync

    def eng_out():
        return nc.scalar

    def ccopy(dst, src):
        cnt["cp"] += 1
        if cnt["cp"] % 2 == 0:
            nc.vector.tensor_copy(dst, src)
        else:
            nc.scalar.copy(dst, src)

    # ---------------- phase 1: repack xs -> table[g], one y-block ----------
    def build_block(g, yb):
        y0 = yb * YB
        rows = min(YB, YS - y0)
        # one DMA per row-shift r covering all 8 channels (slab is f16)
        t4 = tpool.tile([128, 4 * 8 * 179], F16, tag="xsb")
        for r in range(4):
            eng().dma_start(
                bass.AP(t4.tensor, t4.offset + r * 8 * 179,
                        [[t4.ap[0][0], rows], [1, 8 * 179]]),
                bass.AP(xs, (y0 + r) * XS + 128 * g,
                        [[XS, rows], [(YS + 4) * XS, 8], [1, 179]]))
        for h in range(2):
            tg2 = tg2pool.tile([128, XH * 128], F16, tag="tg2")
            for r in range(4):
                # merged interleave+shift:
                # tg2[p, xu*128 + s*32 + r*8 + c] = t[p, c*179 + XH*h + xu + s]
                dst = bass.AP(tg2.tensor, tg2.offset + r * 8,
                              [[tg2.ap[0][0], rows], [32, 4], [128, XH], [1, 8]])
                srcap = bass.AP(t4.tensor, t4.offset + r * 8 * 179 + XH * h,
                                [[t4.ap[0][0], rows], [1, 4], [1, XH], [179, 8]])
                ccopy(dst, srcap)
            # one HWDGE DMA, contiguous 22.5KB per row on both sides
            dsta = bass.AP(tabs[g].tensor,
                           tabs[g].offset + (y0 * XT + h * XH) * 128,
                           [[XT * 128, rows], [1, XH * 128]])
            eng().dma_start(dsta, tg2[:rows, :])

    # ---------------- phase 2: per super-tile weights+idx, gather+combine --
    def cubic(t, tag, outdt, opool):
        # returns w0..w3 tiles [128, SJW] in outdt; all-DVE (no cross-engine
        # handoffs); scratch tags shared between calls (sequential use).
        TS = nc.vector.tensor_scalar
        TT = nc.vector.tensor_tensor
        s0 = wrk.tile([128, SJW], F32, tag="c_s0")
        TS(s0[:], t[:], 1.0, None, op0=OP.add)
        w0f = wrk.tile([128, SJW], F32, tag="c_w0f")
        TS(w0f[:], s0[:], A, -5.0 * A, op0=OP.mult, op1=OP.add)
        TT(w0f[:], w0f[:], s0[:], op=OP.mult)
        TS(w0f[:], w0f[:], 8.0 * A, None, op0=OP.add)
        TT(w0f[:], w0f[:], s0[:], op=OP.mult)
        w0 = opool.tile([128, SJW], outdt, tag=f"w0{tag}")
        TS(w0[:], w0f[:], -4.0 * A, None, op0=OP.add)
        # w1
        w1f = wrk.tile([128, SJW], F32, tag="c_w1f")
        TS(w1f[:], t[:], A + 2.0, -(A + 3.0), op0=OP.mult, op1=OP.add)
        TT(w1f[:], w1f[:], t[:], op=OP.mult)
        TT(w1f[:], w1f[:], t[:], op=OP.mult)
        w1 = opool.tile([128, SJW], outdt, tag=f"w1{tag}")
        TS(w1[:], w1f[:], 1.0, None, op0=OP.add)
        # w2: u = 1 - t
        u = wrk.tile([128, SJW], F32, tag="c_u")
        TS(u[:], t[:], -1.0, 1.0, op0=OP.mult, op1=OP.add)
        w2f = wrk.tile([128, SJW], F32, tag="c_w2f")
        TS(w2f[:], u[:], A + 2.0, -(A + 3.0), op0=OP.mult, op1=OP.add)
        TT(w2f[:], w2f[:], u[:], op=OP.mult)
        TT(w2f[:], w2f[:], u[:], op=OP.mult)
        w2 = opool.tile([128, SJW], outdt, tag=f"w2{tag}")
        TS(w2[:], w2f[:], 1.0, None, op0=OP.add)
        # w3 = 1 - w0 - w1 - w2 (in f32 then cast)
        w3f = wrk.tile([128, SJW], F32, tag="c_w3f")
        TT(w3f[:], w0[:], w1[:], op=OP.add)
        TT(w3f[:], w3f[:], w2[:], op=OP.add)
        w3 = opool.tile([128, SJW], outdt, tag=f"w3{tag}")
        TS(w3[:], w3f[:], -1.0, 1.0, op0=OP.mult, op1=OP.add)
        return [w0, w1, w2, w3]

    def floorpair(v, tag):
        # vi/co scratch shared between calls; vf/fr persist per-dir
        vi = wrk.tile([128, SJW], I32, tag="f_vi")
        nc.vector.tensor_copy(vi[:], v[:])
        vf = wrk.tile([128, SJW], F32, tag=f"vf{tag}")
        nc.vector.tensor_copy(vf[:], vi[:])
        co = wrk.tile([128, SJW], F32, tag="f_co")
        nc.vector.tensor_tensor(co[:], vf[:], v[:], op=OP.is_gt)
        nc.vector.tensor_tensor(vf[:], vf[:], co[:], op=OP.subtract)
        fr = wrk.tile([128, SJW], F32, tag=f"fr{tag}")
        nc.vector.tensor_tensor(fr[:], v[:], vf[:], op=OP.subtract)
        return vf, fr

    def super_tile(g, s4):
        """Weights + wrapped idx for 512 output cols of row-group g.
        Returns (wp2, Cw): wp2[i, (s*4+r)*SJW + j] = wx_s(i,j)*wy_r(i,j)."""
        IG = g * 128
        jb4 = s4 * SJW
        gt = gridp.tile([128, SJW * 2], F32, tag="gt")
        eng_prep().dma_start(
            gt[:],
            bass.AP(gr, IG * W * 2 + jb4 * 2, [[W * 2, 128], [1, SJW * 2]]))
        gx = bass.AP(gt.tensor, gt.offset, [gt.ap[0], [2, SJW]])
        gy = bass.AP(gt.tensor, gt.offset + 1, [gt.ap[0], [2, SJW]])

        lx = wrk.tile([128, SJW], F32, tag="lx")
        ly = wrk.tile([128, SJW], F32, tag="ly")
        nc.vector.tensor_scalar(lx[:], gx, 1024.0, 1047.5 - IG,
                                op0=OP.mult, op1=OP.add)
        nc.vector.tensor_scalar(ly[:], gy, 1024.0, 1046.5 - jb4,
                                op0=OP.mult, op1=OP.add)
        fx, tx = floorpair(lx, "x")
        fy, ty = floorpair(ly, "y")

        # idxf = fy*XT + fx - 1 (f32, exact)
        idxf = wrk.tile([128, SJW], F32, tag="idxf")
        nc.vector.scalar_tensor_tensor(idxf[:], fy[:], float(XT), fx[:],
                                       op0=OP.mult, op1=OP.add)
        # per-sub-tile rebase to the 186-row gather window, cast to i16
        idx16 = idxs1.tile([128, SJW], I16, tag="idx16")
        for t in range(SJW // 128):
            nc.vector.tensor_scalar(
                bass.AP(idx16.tensor, idx16.offset + t * 128,
                        [[idx16.ap[0][0], 128], [1, 128]]),
                bass.AP(idxf.tensor, idxf.offset + t * 128,
                        [[idxf.ap[0][0], 128], [1, 128]]),
                -1.0 - t * 128.0 * XT, None, op0=OP.add)

        # fold [128, SJW] -> wrapped [16, 8*SJW]: D[p, k*SJW + j] = idx16[16k+p, j]
        D = idxs1.tile([128, 8 * SJW], I16, tag="D")
        for k in range(8):
            src = bass.AP(idx16.tensor,
                          idx16.offset + 16 * k * idx16.ap[0][0],
                          [[idx16.ap[0][0], 16], [1, SJW]])
            dst = bass.AP(D.tensor, D.offset + k * SJW,
                          [[D.ap[0][0], 16], [1, SJW]])
            eng_prep().dma_start(dst, src)
        # interleave: Cw[p, 8j+k] = D[p, k*SJW + j]  (one strided copy)
        Cw = idxp.tile([128, 8 * SJW], I16, tag="Cw")
        nc.vector.tensor_copy(
            bass.AP(Cw.tensor, Cw.offset, [[Cw.ap[0][0], 16], [8, SJW], [1, 8]]),
            bass.AP(D.tensor, D.offset, [[D.ap[0][0], 16], [1, SJW], [SJW, 8]]))
        # replicate to all 8 gpsimd cores (doubling: 16 -> 32 -> 64 -> 128)
        for rep in (16, 32, 64):
            src = bass.AP(Cw.tensor, Cw.offset, [[Cw.ap[0][0], rep], [1, 8 * SJW]])
            dst = bass.AP(Cw.tensor, Cw.offset + rep * Cw.ap[0][0],
                          [[Cw.ap[0][0], rep], [1, 8 * SJW]])
            eng_prep().dma_start(dst, src)

        wx = cubic(tx, "x", F16, wrk)
        wy = cubic(ty, "y", F16, wrk)
        # wp2[i, (s*4+r)*SJW + j]: 16 fully-contiguous DVE multiplies
        wp2 = wpp.tile([128, 16 * SJW], F16, tag="wp2")
        for s in range(4):
            for r in range(4):
                dst = bass.AP(wp2.tensor, wp2.offset + (s * 4 + r) * SJW,
                              [wp2.ap[0], [1, SJW]])
                nc.vector.tensor_tensor(dst, wx[s][:], wy[r][:], op=OP.mult)
        return wp2, Cw

    def half_tile(g, s4, t, h, wp2, Cw):
        IG = g * 128
        jb = s4 * SJW + t * 128 + h * JW2
        ybase = s4 * SJW + t * 128
        joff = t * 128 + h * JW2

        # four 2048-idx gathers (ring-resident, 4-queue pipelined):
        # G[i, jl*128 + e], e = s*32 + r*8 + c
        G = gp.tile([128, JW2 * 128], F16, tag="G")
        in_ap = bass.AP(tabs[g].tensor,
                        tabs[g].offset + ybase * XT * 128,
                        [[128, 186 * XT], [1, 128]])
        NSUB = 2048
        for m in range(4):
            idxsl = bass.AP(Cw.tensor, Cw.offset + joff * 8 + m * (NSUB // 16),
                            [[Cw.ap[0][0], 128], [1, NSUB // 16]])
            q = cnt["q"] % 4
            cnt["q"] += 1
            nc.gpsimd.dma_gather(
                out_ap=bass.AP(G.tensor, G.offset + m * 16 * 128,
                               [[G.ap[0][0], 128], [128, 16], [1, 128]]),
                in_ap=in_ap,
                idxs_ap=idxsl,
                num_idxs=NSUB,
                num_idxs_reg=NSUB,
                elem_size=128,
                elem_step=128,
                single_packet=False,
                queue_num=q,
            )

        # combine: G[i, jl*128 + s*32 + r*8 + c] *= wp2[i, (s*4+r)*SJW + jb+jl]
        # (bcast over c) in two halves
        for m in range(2):
            src1 = bass.AP(wp2.tensor, wp2.offset + joff + m * 32,
                           [wp2.ap[0], [1, JW2 // 2], [4 * SJW, 4], [SJW, 4],
                            [0, 8]])
            src0 = bass.AP(G.tensor, G.offset + m * 32 * 128,
                           [G.ap[0], [128, JW2 // 2], [32, 4], [8, 4], [1, 8]])
            nc.vector.tensor_tensor(src0, src0, src1, op=OP.mult)

        def halve(buf, stride, n, tag, npx=JW2):
            o = lp.tile([128, npx * stride * (n // 2)], F16, tag=tag)
            i0 = bass.AP(buf.tensor, buf.offset,
                         [buf.ap[0], [stride * n, npx], [stride * 2, n // 2], [1, stride]])
            i1 = bass.AP(buf.tensor, buf.offset + stride,
                         [buf.ap[0], [stride * n, npx], [stride * 2, n // 2], [1, stride]])
            od = bass.AP(o.tensor, o.offset,
                         [o.ap[0], [stride * (n // 2), npx], [stride, n // 2], [1, stride]])
            nc.vector.tensor_tensor(od, i0, i1, op=OP.add)
            return o

        L1 = halve(G, 32, 4, "L1")
        L2 = halve(L1, 32, 2, "L2")
        L3 = halve(L2, 8, 4, "L3")
        of = outp.tile([128, 8 * JW2], F32, tag="of")
        i0 = bass.AP(L3.tensor, L3.offset, [L3.ap[0], [1, 8], [16, JW2]])
        i1 = bass.AP(L3.tensor, L3.offset + 8, [L3.ap[0], [1, 8], [16, JW2]])
        od = bass.AP(of.tensor, of.offset, [of.ap[0], [JW2, 8], [1, JW2]])
        nc.vector.tensor_tensor(od, i0, i1, op=OP.add)

        # blocked layout [RPC, W//64, C, 64]: 2KB contiguous per partition
        dsto = bass.AP(out, (IG * (W // JW2) + jb // JW2) * C * JW2,
                       [[(W // JW2) * C * JW2, 128], [1, C * JW2]])
        eng_out().dma_start(dsto, of[:])

    # ---------------- emission schedule --------------------------------
    # Serial phases: build both tables first (gather desc-gen on GpSimd
    # thrashes the SBUF ports DVE needs, so overlapping the build with the
    # gather phase slows the interleave copies ~5x). Supers are
    # software-pipelined one ahead so weights compute during gathers.
    for yb in range(N_YB):
        build_block(0, yb)
        build_block(1, yb)

    run_order = [(0, 0), (1, 0), (0, 1), (1, 1), (0, 2), (1, 2), (0, 3), (1, 3)]
    sups = {run_order[0]: super_tile(*run_order[0])}
    for i, (g, s4) in enumerate(run_order):
        if i + 1 < len(run_order):
            gn, sn = run_order[i + 1]
            sups[(gn, sn)] = super_tile(gn, sn)
        for t in range(4):
            for h in range(2):
                half_tile(g, s4, t, h, *sups[(g, s4)])
        del sups[(g, s4)]


_NC_CACHE = None


def kernel(x: np.ndarray, grid: np.ndarray) -> np.ndarray:
    global _NC_CACHE
    if _NC_CACHE is None:
        _NC_CACHE = build_nc()
    nc = _NC_CACHE

    x0 = np.ascontiguousarray(x[0], dtype=np.float32)        # [C, H, W]
    g0 = np.ascontiguousarray(grid[0], dtype=np.float32)     # [H, W, 2]

    in_maps = []
    for k in range(N_CORES):
        I0 = k * RPC
        xsl = np.zeros((C, YS + 4, XS), dtype=np.float16)
        c0 = I0 - PAD
        lo, hi = max(0, c0), min(W, c0 + XS)
        xsl[:, PAD:PAD + H, lo - c0:hi - c0] = x0[:, :, lo:hi].astype(np.float16)
        grc = np.ascontiguousarray(g0[I0:I0 + RPC]).copy()
        grc[..., 0] -= I0 / 1024.0   # fold per-core x-base into gx
        in_maps.append({"xs": xsl, "gr": grc})

    res = run_bass_kernel_spmd(nc, in_maps, core_ids=list(range(N_CORES)),
                               trace=False)
    global _LAST_EXEC_NS, _LAST_RES
    _LAST_EXEC_NS = res.exec_time_ns
    _LAST_RES = res
    out = np.empty((1, C, H, W), dtype=np.float32)
    for k in range(N_CORES):
        blk = res.results[k]["out"]          # [RPC, W//64, C, 64]
        out[0, :, k * RPC:(k + 1) * RPC, :] = (
            blk.transpose(2, 0, 1, 3).reshape(C, RPC, W))
    return out


# revision 11
# speedup vs baseline: 1.2480x; 1.0009x over previous
"""Bicubic grid_sample (transpose-like warp) for Trainium2, 8 NeuronCores.

Strategy: shard output rows across cores (256 rows/core). The warp maps
output (i, j) -> input (y ~ j +- 21, x ~ i +- 21), so each core needs an
x-column slab of the image. On device, repack the slab into a patch table
in DRAM where each 256B unit holds the full 4x4x8ch bicubic patch at
(y0, x0) (fp16). v3: slab ships as f16 (halves slab DMA), gathers merged
to one 8192-idx call per 64-col half-tile (4x fewer calls), weight
products built with fully-contiguous (s,r)-major ops (no strided ACT
copies), and row-group 1's table build is emitted interleaved with
row-group 0's gather/combine so the build streams through the latency
bubbles of the gather phase.
"""
import os, sys, types
sys.path.insert(0, "/opt/trn_rl_repo")
import numpy as np

try:  # register NTFF profile hook so BASS_TRACE=1 can measure HW time
    import antenv
    if "antenv.axon_hooks" not in sys.modules:
        from trn_agent_boot.trn_boot import _ntff_profile_via_ctypes
        _h = _ntff_profile_via_ctypes("/opt/axon/libaxon_pjrt.so")
        _m = types.ModuleType("antenv.axon_hooks")
        _m.get_axon_ntff_profile_hook = lambda: _h
        _m.set_axon_ntff_profile_hook = lambda h: None
        sys.modules["antenv.axon_hooks"] = _m
        antenv.axon_hooks = _m
except Exception:
    pass

import concourse.bass as bass
import concourse.bacc as bacc
import concourse.mybir as mybir
import concourse.tile as tile
from concourse import library_config
from concourse.bass_utils import run_bass_kernel_spmd

F32 = mybir.dt.float32
F16 = mybir.dt.float16
I16 = mybir.dt.int16
I32 = mybir.dt.int32
OP = mybir.AluOpType

N_CORES = 8
H = W = 2048
C = 8
RPC = H // N_CORES          # output rows per core = 256
PAD = 24                    # y halo rows on each side
YS = H + 2 * PAD            # 2096 slab rows
XS = 308                    # slab cols: [I0-24, I0+284)
XT = 176                    # table cols per row-group
XH = 88                     # x-half of the table staging buffer
YT = YS + 16                # table rows incl. pad so in_ap window stays in-bounds
SJW = 512                   # super-tile width (weights/idx granularity)
JW2 = 64                    # half-tile width (gather/combine granularity)
A = -0.75                   # bicubic constant
YB = 124                    # y-block rows for table build
N_YB = (YS + YB - 1) // YB  # 17


def build_nc():
    nc = bacc.Bacc("TRN2", target_bir_lowering=False, debug=False,
                   num_devices=N_CORES, num_swdge_queues=4)
    xs = nc.dram_tensor("xs", [C, YS + 4, XS], F16, kind="ExternalInput")
    gr = nc.dram_tensor("gr", [RPC, W, 2], F32, kind="ExternalInput")
    out = nc.dram_tensor("out", [RPC, W // JW2, C, JW2], F32,
                         kind="ExternalOutput")

    with tile.TileContext(nc) as tc:
        nc.gpsimd.load_library(library_config.mlp)
        import contextlib
        with contextlib.ExitStack() as ctx:
            _build_body(ctx, tc, nc, xs, gr, out)
    nc.compile()
    return nc


def _build_body(ctx, tc, nc, xs, gr, out):
    tabpool = ctx.enter_context(tc.tile_pool(name="tab", bufs=1, space="DRAM"))
    # phase-1 pools
    tpool = ctx.enter_context(tc.tile_pool(name="t", bufs=2))
    tg2pool = ctx.enter_context(tc.tile_pool(name="tg2", bufs=1))
    # phase-2 pools
    gridp = ctx.enter_context(tc.tile_pool(name="grid", bufs=2))
    wrk = ctx.enter_context(tc.tile_pool(name="wrk", bufs=1))
    wpp = ctx.enter_context(tc.tile_pool(name="wpp", bufs=2))
    idxp = ctx.enter_context(tc.tile_pool(name="idx", bufs=2))
    idxs1 = ctx.enter_context(tc.tile_pool(name="idx1", bufs=1))
    gp = ctx.enter_context(tc.tile_pool(name="g", bufs=2))
    lp = ctx.enter_context(tc.tile_pool(name="l", bufs=1))
    outp = ctx.enter_context(tc.tile_pool(name="out", bufs=2))

    tabs = []
    for g in range(2):
        tabg = tabpool.tile([YT * XT, 128], F16, tag=f"tab{g}")
        tabs.append(tabg)

    hwdge = [nc.sync, nc.scalar]
    cnt = {"dma": 0, "cp": 0, "q": 0}

    def eng():
        # phase-1 bulk traffic: alternate both HWDGE engines
        cnt["dma"] += 1
        return hwdge[cnt["dma"] % 2]

    def eng_prep():
        # phase-2 latency-critical prep (grid, idx folds, replication):
        # keep off the output-write queue to avoid head-of-line blocking
        return nc.sync

    def eng_out():
        return nc.scalar

    def ccopy(dst, src):
        cnt["cp"] += 1
        if cnt["cp"] % 2 == 0:
            nc.vector.tensor_copy(dst, src)
        else:
            nc.scalar.copy(dst, src)

    # ---------------- phase 1: repack xs -> table[g], one y-block ----------
    def build_block(g, yb):
        y0 = yb * YB
        rows = min(YB, YS - y0)
        # one DMA per row-shift r covering all 8 channels (slab is f16)
        t4 = tpool.tile([128, 4 * 8 * 179], F16, tag="xsb")
        for r in range(4):
            eng().dma_start(
                bass.AP(t4.tensor, t4.offset + r * 8 * 179,
                        [[t4.ap[0][0], rows], [1, 8 * 179]]),
                bass.AP(xs, (y0 + r) * XS + 128 * g,
                        [[XS, rows], [(YS + 4) * XS, 8], [1, 179]]))
        for h in range(2):
            tg2 = tg2pool.tile([128, XH * 128], F16, tag="tg2")
            for r in range(4):
                # merged interleave+shift:
                # tg2[p, xu*128 + s*32 + r*8 + c] = t[p, c*179 + XH*h + xu + s]
                dst = bass.AP(tg2.tensor, tg2.offset + r * 8,
                              [[tg2.ap[0][0], rows], [32, 4], [128, XH], [1, 8]])
                srcap = bass.AP(t4.tensor, t4.offset + r * 8 * 179 + XH * h,
                                [[t4.ap[0][0], rows], [1, 4], [1, XH], [179, 8]])
                ccopy(dst, srcap)
            # one HWDGE DMA, contiguous 22.5KB per row on both sides
            dsta = bass.AP(tabs[g].tensor,
                           tabs[g].offset + (y0 * XT + h * XH) * 128,
                           [[XT * 128, rows], [1, XH * 128]])
            eng().dma_start(dsta, tg2[:rows, :])

    # ---------------- phase 2: per super-tile weights+idx, gather+combine --
    def cubic(t, tag, outdt, opool):
        # returns w0..w3 tiles [128, SJW] in outdt; all-DVE (no cross-engine
        # handoffs); scratch tags shared between calls (sequential use).
        TS = nc.vector.tensor_scalar
        TT = nc.vector.tensor_tensor
        s0 = wrk.tile([128, SJW], F32, tag="c_s0")
        TS(s0[:], t[:], 1.0, None, op0=OP.add)
        w0f = wrk.tile([128, SJW], F32, tag="c_w0f")
        TS(w0f[:], s0[:], A, -5.0 * A, op0=OP.mult, op1=OP.add)
        TT(w0f[:], w0f[:], s0[:], op=OP.mult)
        TS(w0f[:], w0f[:], 8.0 * A, None, op0=OP.add)
        TT(w0f[:], w0f[:], s0[:], op=OP.mult)
        w0 = opool.tile([128, SJW], outdt, tag=f"w0{tag}")
        TS(w0[:], w0f[:], -4.0 * A, None, op0=OP.add)
        # w1
        w1f = wrk.tile([128, SJW], F32, tag="c_w1f")
        TS(w1f[:], t[:], A + 2.0, -(A + 3.0), op0=OP.mult, op1=OP.add)
        TT(w1f[:], w1f[:], t[:], op=OP.mult)
        TT(w1f[:], w1f[:], t[:], op=OP.mult)
        w1 = opool.tile([128, SJW], outdt, tag=f"w1{tag}")
        TS(w1[:], w1f[:], 1.0, None, op0=OP.add)
        # w2: u = 1 - t
        u = wrk.tile([128, SJW], F32, tag="c_u")
        TS(u[:], t[:], -1.0, 1.0, op0=OP.mult, op1=OP.add)
        w2f = wrk.tile([128, SJW], F32, tag="c_w2f")
        TS(w2f[:], u[:], A + 2.0, -(A + 3.0), op0=OP.mult, op1=OP.add)
        TT(w2f[:], w2f[:], u[:], op=OP.mult)
        TT(w2f[:], w2f[:], u[:], op=OP.mult)
        w2 = opool.tile([128, SJW], outdt, tag=f"w2{tag}")
        TS(w2[:], w2f[:], 1.0, None, op0=OP.add)
        # w3 = 1 - w0 - w1 - w2 (in f32 then cast)
        w3f = wrk.tile([128, SJW], F32, tag="c_w3f")
        TT(w3f[:], w0[:], w1[:], op=OP.add)
        TT(w3f[:], w3f[:], w2[:], op=OP.add)
        w3 = opool.tile([128, SJW], outdt, tag=f"w3{tag}")
        TS(w3[:], w3f[:], -1.0, 1.0, op0=OP.mult, op1=OP.add)
        return [w0, w1, w2, w3]

    def floorpair(v, tag):
        # vi/co scratch shared between calls; vf/fr persist per-dir
        vi = wrk.tile([128, SJW], I32, tag="f_vi")
        nc.vector.tensor_copy(vi[:], v[:])
        vf = wrk.tile([128, SJW], F32, tag=f"vf{tag}")
        nc.vector.tensor_copy(vf[:], vi[:])
        co = wrk.tile([128, SJW], F32, tag="f_co")
        nc.vector.tensor_tensor(co[:], vf[:], v[:], op=OP.is_gt)
        nc.vector.tensor_tensor(vf[:], vf[:], co[:], op=OP.subtract)
        fr = wrk.tile([128, SJW], F32, tag=f"fr{tag}")
        nc.vector.tensor_tensor(fr[:], v[:], vf[:], op=OP.subtract)
        return vf, fr

    def super_tile(g, s4):
        """Weights + wrapped idx for 512 output cols of row-group g.
        Returns (wp2, Cw): wp2[i, (s*4+r)*SJW + j] = wx_s(i,j)*wy_r(i,j)."""
        IG = g * 128
        jb4 = s4 * SJW
        gt = gridp.tile([128, SJW * 2], F32, tag="gt")
        eng_prep().dma_start(
            gt[:],
            bass.AP(gr, IG * W * 2 + jb4 * 2, [[W * 2, 128], [1, SJW * 2]]))
        gx = bass.AP(gt.tensor, gt.offset, [gt.ap[0], [2, SJW]])
        gy = bass.AP(gt.tensor, gt.offset + 1, [gt.ap[0], [2, SJW]])

        lx = wrk.tile([128, SJW], F32, tag="lx")
        ly = wrk.tile([128, SJW], F32, tag="ly")
        nc.vector.tensor_scalar(lx[:], gx, 1024.0, 1047.5 - IG,
                                op0=OP.mult, op1=OP.add)
        nc.vector.tensor_scalar(ly[:], gy, 1024.0, 1046.5 - jb4,
                                op0=OP.mult, op1=OP.add)
        fx, tx = floorpair(lx, "x")
        fy, ty = floorpair(ly, "y")

        # idxf = fy*XT + fx - 1 (f32, exact)
        idxf = wrk.tile([128, SJW], F32, tag="idxf")
        nc.vector.scalar_tensor_tensor(idxf[:], fy[:], float(XT), fx[:],
                                       op0=OP.mult, op1=OP.add)
        # per-sub-tile rebase to the 186-row gather window, cast to i16
        idx16 = idxs1.tile([128, SJW], I16, tag="idx16")
        for t in range(SJW // 128):
            nc.vector.tensor_scalar(
                bass.AP(idx16.tensor, idx16.offset + t * 128,
                        [[idx16.ap[0][0], 128], [1, 128]]),
                bass.AP(idxf.tensor, idxf.offset + t * 128,
                        [[idxf.ap[0][0], 128], [1, 128]]),
                -1.0 - t * 128.0 * XT, None, op0=OP.add)

        # fold [128, SJW] -> wrapped [16, 8*SJW]: D[p, k*SJW + j] = idx16[16k+p, j]
        D = idxs1.tile([128, 8 * SJW], I16, tag="D")
        for k in range(8):
            src = bass.AP(idx16.tensor,
                          idx16.offset + 16 * k * idx16.ap[0][0],
                          [[idx16.ap[0][0], 16], [1, SJW]])
            dst = bass.AP(D.tensor, D.offset + k * SJW,
                          [[D.ap[0][0], 16], [1, SJW]])
            eng_prep().dma_start(dst, src)
        # replicate D to all 8 gpsimd core bands (7 independent copies, no
        # chaining), then one full-width interleave: Cw[p, 8j+k] = D[p%16, k*SJW+j]
        for rep in range(1, 8):
            src = bass.AP(D.tensor, D.offset, [[D.ap[0][0], 16], [1, 8 * SJW]])
            dst = bass.AP(D.tensor, D.offset + 16 * rep * D.ap[0][0],
                          [[D.ap[0][0], 16], [1, 8 * SJW]])
            eng_prep().dma_start(dst, src)
        Cw = idxp.tile([128, 8 * SJW], I16, tag="Cw")
        nc.vector.tensor_copy(
            bass.AP(Cw.tensor, Cw.offset, [[Cw.ap[0][0], 128], [8, SJW], [1, 8]]),
            bass.AP(D.tensor, D.offset, [[D.ap[0][0], 128], [1, SJW], [SJW, 8]]))

        wx = cubic(tx, "x", F16, wrk)
        wy = cubic(ty, "y", F16, wrk)
        # wp2[i, (s*4+r)*SJW + j]: 16 fully-contiguous DVE multiplies
        wp2 = wpp.tile([128, 16 * SJW], F16, tag="wp2")
        for s in range(4):
            for r in range(4):
                dst = bass.AP(wp2.tensor, wp2.offset + (s * 4 + r) * SJW,
                              [wp2.ap[0], [1, SJW]])
                nc.vector.tensor_tensor(dst, wx[s][:], wy[r][:], op=OP.mult)
        return wp2, Cw

    def half_tile(g, s4, t, h, wp2, Cw):
        IG = g * 128
        jb = s4 * SJW + t * 128 + h * JW2
        ybase = s4 * SJW + t * 128
        joff = t * 128 + h * JW2

        # four 2048-idx gathers (ring-resident, 4-queue pipelined):
        # G[i, jl*128 + e], e = s*32 + r*8 + c
        G = gp.tile([128, JW2 * 128], F16, tag="G")
        in_ap = bass.AP(tabs[g].tensor,
                        tabs[g].offset + ybase * XT * 128,
                        [[128, 186 * XT], [1, 128]])
        NSUB = 2048
        for m in range(4):
            idxsl = bass.AP(Cw.tensor, Cw.offset + joff * 8 + m * (NSUB // 16),
                            [[Cw.ap[0][0], 128], [1, NSUB // 16]])
            q = cnt["q"] % 4
            cnt["q"] += 1
            nc.gpsimd.dma_gather(
                out_ap=bass.AP(G.tensor, G.offset + m * 16 * 128,
                               [[G.ap[0][0], 128], [128, 16], [1, 128]]),
                in_ap=in_ap,
                idxs_ap=idxsl,
                num_idxs=NSUB,
                num_idxs_reg=NSUB,
                elem_size=128,
                elem_step=128,
                single_packet=False,
                queue_num=q,
            )

        # combine: G[i, jl*128 + s*32 + r*8 + c] *= wp2[i, (s*4+r)*SJW + jb+jl]
        # (bcast over c) in two halves
        for m in range(2):
            src1 = bass.AP(wp2.tensor, wp2.offset + joff + m * 32,
                           [wp2.ap[0], [1, JW2 // 2], [4 * SJW, 4], [SJW, 4],
                            [0, 8]])
            src0 = bass.AP(G.tensor, G.offset + m * 32 * 128,
                           [G.ap[0], [128, JW2 // 2], [32, 4], [8, 4], [1, 8]])
            nc.vector.tensor_tensor(src0, src0, src1, op=OP.mult)

        def halve(buf, stride, n, tag, npx=JW2):
            o = lp.tile([128, npx * stride * (n // 2)], F16, tag=tag)
            i0 = bass.AP(buf.tensor, buf.offset,
                         [buf.ap[0], [stride * n, npx], [stride * 2, n // 2], [1, stride]])
            i1 = bass.AP(buf.tensor, buf.offset + stride,
                         [buf.ap[0], [stride * n, npx], [stride * 2, n // 2], [1, stride]])
            od = bass.AP(o.tensor, o.offset,
                         [o.ap[0], [stride * (n // 2), npx], [stride, n // 2], [1, stride]])
            nc.vector.tensor_tensor(od, i0, i1, op=OP.add)
            return o

        L1 = halve(G, 32, 4, "L1")
        L2 = halve(L1, 32, 2, "L2")
        L3 = halve(L2, 8, 4, "L3")
        of = outp.tile([128, 8 * JW2], F32, tag="of")
        i0 = bass.AP(L3.tensor, L3.offset, [L3.ap[0], [1, 8], [16, JW2]])
        i1 = bass.AP(L3.tensor, L3.offset + 8, [L3.ap[0], [1, 8], [16, JW2]])
        od = bass.AP(of.tensor, of.offset, [of.ap[0], [JW2, 8], [1, JW2]])
        nc.vector.tensor_tensor(od, i0, i1, op=OP.add)

        # blocked layout [RPC, W//64, C, 64]: 2KB contiguous per partition
        dsto = bass.AP(out, (IG * (W // JW2) + jb // JW2) * C * JW2,
                       [[(W // JW2) * C * JW2, 128], [1, C * JW2]])
        eng_out().dma_start(dsto, of[:])

    # ---------------- emission schedule --------------------------------
    # Serial phases: build both tables first (gather desc-gen on GpSimd
    # thrashes the SBUF ports DVE needs, so overlapping the build with the
    # gather phase slows the interleave copies ~5x). Supers are
    # software-pipelined one ahead so weights compute during gathers.
    for yb in range(N_YB):
        build_block(0, yb)
        build_block(1, yb)

    run_order = [(0, 0), (1, 0), (0, 1), (1, 1), (0, 2), (1, 2), (0, 3), (1, 3)]
    sups = {run_order[0]: super_tile(*run_order[0])}
    for i, (g, s4) in enumerate(run_order):
        if i + 1 < len(run_order):
            gn, sn = run_order[i + 1]
            sups[(gn, sn)] = super_tile(gn, sn)
        for t in range(4):
            for h in range(2):
                half_tile(g, s4, t, h, *sups[(g, s4)])
        del sups[(g, s4)]


_NC_CACHE = None


def kernel(x: np.ndarray, grid: np.ndarray) -> np.ndarray:
    global _NC_CACHE
    if _NC_CACHE is None:
        _NC_CACHE = build_nc()
    nc = _NC_CACHE

    x0 = np.ascontiguousarray(x[0], dtype=np.float32)        # [C, H, W]
    g0 = np.ascontiguousarray(grid[0], dtype=np.float32)     # [H, W, 2]

    in_maps = []
    for k in range(N_CORES):
        I0 = k * RPC
        xsl = np.zeros((C, YS + 4, XS), dtype=np.float16)
        c0 = I0 - PAD
        lo, hi = max(0, c0), min(W, c0 + XS)
        xsl[:, PAD:PAD + H, lo - c0:hi - c0] = x0[:, :, lo:hi].astype(np.float16)
        grc = np.ascontiguousarray(g0[I0:I0 + RPC]).copy()
        grc[..., 0] -= I0 / 1024.0   # fold per-core x-base into gx
        in_maps.append({"xs": xsl, "gr": grc})

    res = run_bass_kernel_spmd(nc, in_maps, core_ids=list(range(N_CORES)),
                               trace=False)
    global _LAST_EXEC_NS, _LAST_RES
    _LAST_EXEC_NS = res.exec_time_ns
    _LAST_RES = res
    out = np.empty((1, C, H, W), dtype=np.float32)
    for k in range(N_CORES):
        blk = res.results[k]["out"]          # [RPC, W//64, C, 64]
        out[0, :, k * RPC:(k + 1) * RPC, :] = (
            blk.transpose(2, 0, 1, 3).reshape(C, RPC, W))
    return out


# revision 12
# speedup vs baseline: 1.2843x; 1.0291x over previous
"""Bicubic grid_sample (transpose-like warp) for Trainium2, 8 NeuronCores.

Strategy: shard output rows across cores (256 rows/core). The warp maps
output (i, j) -> input (y ~ j +- 21, x ~ i +- 21), so each core needs an
x-column slab of the image. On device, repack the slab into a patch table
in DRAM where each 256B unit holds the full 4x4x8ch bicubic patch at
(y0, x0) (fp16). v3: slab ships as f16 (halves slab DMA), gathers merged
to one 8192-idx call per 64-col half-tile (4x fewer calls), weight
products built with fully-contiguous (s,r)-major ops (no strided ACT
copies), and row-group 1's table build is emitted interleaved with
row-group 0's gather/combine so the build streams through the latency
bubbles of the gather phase.
"""
import os, sys, types
sys.path.insert(0, "/opt/trn_rl_repo")
import numpy as np

try:  # register NTFF profile hook so BASS_TRACE=1 can measure HW time
    import antenv
    if "antenv.axon_hooks" not in sys.modules:
        from trn_agent_boot.trn_boot import _ntff_profile_via_ctypes
        _h = _ntff_profile_via_ctypes("/opt/axon/libaxon_pjrt.so")
        _m = types.ModuleType("antenv.axon_hooks")
        _m.get_axon_ntff_profile_hook = lambda: _h
        _m.set_axon_ntff_profile_hook = lambda h: None
        sys.modules["antenv.axon_hooks"] = _m
        antenv.axon_hooks = _m
except Exception:
    pass

import concourse.bass as bass
import concourse.bacc as bacc
import concourse.mybir as mybir
import concourse.tile as tile
from concourse import library_config
from concourse.bass_utils import run_bass_kernel_spmd

F32 = mybir.dt.float32
F16 = mybir.dt.float16
I16 = mybir.dt.int16
I32 = mybir.dt.int32
OP = mybir.AluOpType

N_CORES = 8
H = W = 2048
C = 8
RPC = H // N_CORES          # output rows per core = 256
PAD = 24                    # y halo rows on each side
YS = H + 2 * PAD            # 2096 slab rows
XS = 308                    # slab cols: [I0-24, I0+284)
XT = 176                    # table cols per row-group
XH = 88                     # x-half of the table staging buffer
YT = YS + 16                # table rows incl. pad so in_ap window stays in-bounds
SJW = 512                   # super-tile width (weights/idx granularity)
JW2 = 64                    # half-tile width (gather/combine granularity)
A = -0.75                   # bicubic constant
YB = 124                    # y-block rows for table build
N_YB = (YS + YB - 1) // YB  # 17


def build_nc():
    nc = bacc.Bacc("TRN2", target_bir_lowering=False, debug=False,
                   num_devices=N_CORES, num_swdge_queues=4)
    xs = nc.dram_tensor("xs", [C, YS + 4, XS], F16, kind="ExternalInput")
    gr = nc.dram_tensor("gr", [RPC, W, 2], F32, kind="ExternalInput")
    out = nc.dram_tensor("out", [RPC, W // JW2, C, JW2], F32,
                         kind="ExternalOutput")

    with tile.TileContext(nc) as tc:
        nc.gpsimd.load_library(library_config.mlp)
        import contextlib
        with contextlib.ExitStack() as ctx:
            _build_body(ctx, tc, nc, xs, gr, out)
    nc.compile()
    return nc


def _build_body(ctx, tc, nc, xs, gr, out):
    tabpool = ctx.enter_context(tc.tile_pool(name="tab", bufs=1, space="DRAM"))
    # phase-1 pools
    tpool = ctx.enter_context(tc.tile_pool(name="t", bufs=2))
    tg2pool = ctx.enter_context(tc.tile_pool(name="tg2", bufs=1))
    # phase-2 pools
    gridp = ctx.enter_context(tc.tile_pool(name="grid", bufs=2))
    wrk = ctx.enter_context(tc.tile_pool(name="wrk", bufs=1))
    wpp = ctx.enter_context(tc.tile_pool(name="wpp", bufs=2))
    idxp = ctx.enter_context(tc.tile_pool(name="idx", bufs=2))
    idxs1 = ctx.enter_context(tc.tile_pool(name="idx1", bufs=1))
    gp = ctx.enter_context(tc.tile_pool(name="g", bufs=2))
    lp = ctx.enter_context(tc.tile_pool(name="l", bufs=1))
    outp = ctx.enter_context(tc.tile_pool(name="out", bufs=2))

    tabs = []
    for g in range(2):
        tabg = tabpool.tile([YT * XT, 128], F16, tag=f"tab{g}")
        tabs.append(tabg)

    hwdge = [nc.sync, nc.scalar]
    cnt = {"dma": 0, "cp": 0, "q": 0}

    def eng():
        # phase-1 bulk traffic: alternate both HWDGE engines
        cnt["dma"] += 1
        return hwdge[cnt["dma"] % 2]

    def eng_prep():
        # phase-2 small DMAs go through SWDGE (gpsimd): the HWDGE queues are
        # backed up for ~1ms draining phase-1's table writes, and their
        # in-order FIFO would stall prep/output behind that drain.
        return nc.gpsimd

    def eng_out():
        return nc.gpsimd

    def ccopy(dst, src):
        cnt["cp"] += 1
        if cnt["cp"] % 2 == 0:
            nc.vector.tensor_copy(dst, src)
        else:
            nc.scalar.copy(dst, src)

    # ---------------- phase 1: repack xs -> table[g], one y-block ----------
    def build_block(g, yb):
        y0 = yb * YB
        rows = min(YB, YS - y0)
        # one DMA per row-shift r covering all 8 channels (slab is f16)
        t4 = tpool.tile([128, 4 * 8 * 179], F16, tag="xsb")
        for r in range(4):
            eng().dma_start(
                bass.AP(t4.tensor, t4.offset + r * 8 * 179,
                        [[t4.ap[0][0], rows], [1, 8 * 179]]),
                bass.AP(xs, (y0 + r) * XS + 128 * g,
                        [[XS, rows], [(YS + 4) * XS, 8], [1, 179]]))
        for h in range(2):
            tg2 = tg2pool.tile([128, XH * 128], F16, tag="tg2")
            for r in range(4):
                # merged interleave+shift:
                # tg2[p, xu*128 + s*32 + r*8 + c] = t[p, c*179 + XH*h + xu + s]
                dst = bass.AP(tg2.tensor, tg2.offset + r * 8,
                              [[tg2.ap[0][0], rows], [32, 4], [128, XH], [1, 8]])
                srcap = bass.AP(t4.tensor, t4.offset + r * 8 * 179 + XH * h,
                                [[t4.ap[0][0], rows], [1, 4], [1, XH], [179, 8]])
                ccopy(dst, srcap)
            # one HWDGE DMA, contiguous 22.5KB per row on both sides
            dsta = bass.AP(tabs[g].tensor,
                           tabs[g].offset + (y0 * XT + h * XH) * 128,
                           [[XT * 128, rows], [1, XH * 128]])
            eng().dma_start(dsta, tg2[:rows, :])

    # ---------------- phase 2: per super-tile weights+idx, gather+combine --
    def cubic(t, tag, outdt, opool):
        # returns w0..w3 tiles [128, SJW] in outdt; all-DVE (no cross-engine
        # handoffs); scratch tags shared between calls (sequential use).
        TS = nc.vector.tensor_scalar
        TT = nc.vector.tensor_tensor
        s0 = wrk.tile([128, SJW], F32, tag="c_s0")
        TS(s0[:], t[:], 1.0, None, op0=OP.add)
        w0f = wrk.tile([128, SJW], F32, tag="c_w0f")
        TS(w0f[:], s0[:], A, -5.0 * A, op0=OP.mult, op1=OP.add)
        TT(w0f[:], w0f[:], s0[:], op=OP.mult)
        TS(w0f[:], w0f[:], 8.0 * A, None, op0=OP.add)
        TT(w0f[:], w0f[:], s0[:], op=OP.mult)
        w0 = opool.tile([128, SJW], outdt, tag=f"w0{tag}")
        TS(w0[:], w0f[:], -4.0 * A, None, op0=OP.add)
        # w1
        w1f = wrk.tile([128, SJW], F32, tag="c_w1f")
        TS(w1f[:], t[:], A + 2.0, -(A + 3.0), op0=OP.mult, op1=OP.add)
        TT(w1f[:], w1f[:], t[:], op=OP.mult)
        TT(w1f[:], w1f[:], t[:], op=OP.mult)
        w1 = opool.tile([128, SJW], outdt, tag=f"w1{tag}")
        TS(w1[:], w1f[:], 1.0, None, op0=OP.add)
        # w2: u = 1 - t
        u = wrk.tile([128, SJW], F32, tag="c_u")
        TS(u[:], t[:], -1.0, 1.0, op0=OP.mult, op1=OP.add)
        w2f = wrk.tile([128, SJW], F32, tag="c_w2f")
        TS(w2f[:], u[:], A + 2.0, -(A + 3.0), op0=OP.mult, op1=OP.add)
        TT(w2f[:], w2f[:], u[:], op=OP.mult)
        TT(w2f[:], w2f[:], u[:], op=OP.mult)
        w2 = opool.tile([128, SJW], outdt, tag=f"w2{tag}")
        TS(w2[:], w2f[:], 1.0, None, op0=OP.add)
        # w3 = 1 - w0 - w1 - w2 (in f32 then cast)
        w3f = wrk.tile([128, SJW], F32, tag="c_w3f")
        TT(w3f[:], w0[:], w1[:], op=OP.add)
        TT(w3f[:], w3f[:], w2[:], op=OP.add)
        w3 = opool.tile([128, SJW], outdt, tag=f"w3{tag}")
        TS(w3[:], w3f[:], -1.0, 1.0, op0=OP.mult, op1=OP.add)
        return [w0, w1, w2, w3]

    def floorpair(v, tag):
        # vi/co scratch shared between calls; vf/fr persist per-dir
        vi = wrk.tile([128, SJW], I32, tag="f_vi")
        nc.vector.tensor_copy(vi[:], v[:])
        vf = wrk.tile([128, SJW], F32, tag=f"vf{tag}")
        nc.vector.tensor_copy(vf[:], vi[:])
        co = wrk.tile([128, SJW], F32, tag="f_co")
        nc.vector.tensor_tensor(co[:], vf[:], v[:], op=OP.is_gt)
        nc.vector.tensor_tensor(vf[:], vf[:], co[:], op=OP.subtract)
        fr = wrk.tile([128, SJW], F32, tag=f"fr{tag}")
        nc.vector.tensor_tensor(fr[:], v[:], vf[:], op=OP.subtract)
        return vf, fr

    def super_tile(g, s4):
        """Weights + wrapped idx for 512 output cols of row-group g.
        Returns (wp2, Cw): wp2[i, (s*4+r)*SJW + j] = wx_s(i,j)*wy_r(i,j)."""
        IG = g * 128
        jb4 = s4 * SJW
        gt = gridp.tile([128, SJW * 2], F32, tag="gt")
        nc.gpsimd.dma_start(
            gt[:],
            bass.AP(gr, IG * W * 2 + jb4 * 2, [[W * 2, 128], [1, SJW * 2]]))
        gx = bass.AP(gt.tensor, gt.offset, [gt.ap[0], [2, SJW]])
        gy = bass.AP(gt.tensor, gt.offset + 1, [gt.ap[0], [2, SJW]])

        lx = wrk.tile([128, SJW], F32, tag="lx")
        ly = wrk.tile([128, SJW], F32, tag="ly")
        nc.vector.tensor_scalar(lx[:], gx, 1024.0, 1047.5 - IG,
                                op0=OP.mult, op1=OP.add)
        nc.vector.tensor_scalar(ly[:], gy, 1024.0, 1046.5 - jb4,
                                op0=OP.mult, op1=OP.add)
        fx, tx = floorpair(lx, "x")
        fy, ty = floorpair(ly, "y")

        # idxf = fy*XT + fx - 1 (f32, exact)
        idxf = wrk.tile([128, SJW], F32, tag="idxf")
        nc.vector.scalar_tensor_tensor(idxf[:], fy[:], float(XT), fx[:],
                                       op0=OP.mult, op1=OP.add)
        # per-sub-tile rebase to the 186-row gather window, cast to i16
        idx16 = idxs1.tile([128, SJW], I16, tag="idx16")
        for t in range(SJW // 128):
            nc.vector.tensor_scalar(
                bass.AP(idx16.tensor, idx16.offset + t * 128,
                        [[idx16.ap[0][0], 128], [1, 128]]),
                bass.AP(idxf.tensor, idxf.offset + t * 128,
                        [[idxf.ap[0][0], 128], [1, 128]]),
                -1.0 - t * 128.0 * XT, None, op0=OP.add)

        # fold [128, SJW] -> wrapped [16, 8*SJW]: D[p, k*SJW + j] = idx16[16k+p, j]
        D = idxs1.tile([128, 8 * SJW], I16, tag="D")
        for k in range(8):
            src = bass.AP(idx16.tensor,
                          idx16.offset + 16 * k * idx16.ap[0][0],
                          [[idx16.ap[0][0], 16], [1, SJW]])
            dst = bass.AP(D.tensor, D.offset + k * SJW,
                          [[D.ap[0][0], 16], [1, SJW]])
            eng_prep().dma_start(dst, src)
        # replicate D to all 8 gpsimd core bands (7 independent copies, no
        # chaining), then one full-width interleave: Cw[p, 8j+k] = D[p%16, k*SJW+j]
        for rep in range(1, 8):
            src = bass.AP(D.tensor, D.offset, [[D.ap[0][0], 16], [1, 8 * SJW]])
            dst = bass.AP(D.tensor, D.offset + 16 * rep * D.ap[0][0],
                          [[D.ap[0][0], 16], [1, 8 * SJW]])
            eng_prep().dma_start(dst, src)
        Cw = idxp.tile([128, 8 * SJW], I16, tag="Cw")
        nc.vector.tensor_copy(
            bass.AP(Cw.tensor, Cw.offset, [[Cw.ap[0][0], 128], [8, SJW], [1, 8]]),
            bass.AP(D.tensor, D.offset, [[D.ap[0][0], 128], [1, SJW], [SJW, 8]]))

        wx = cubic(tx, "x", F16, wrk)
        wy = cubic(ty, "y", F16, wrk)
        # wp2[i, (s*4+r)*SJW + j]: 16 fully-contiguous DVE multiplies
        wp2 = wpp.tile([128, 16 * SJW], F16, tag="wp2")
        for s in range(4):
            for r in range(4):
                dst = bass.AP(wp2.tensor, wp2.offset + (s * 4 + r) * SJW,
                              [wp2.ap[0], [1, SJW]])
                nc.vector.tensor_tensor(dst, wx[s][:], wy[r][:], op=OP.mult)
        return wp2, Cw

    def half_tile(g, s4, t, h, wp2, Cw):
        IG = g * 128
        jb = s4 * SJW + t * 128 + h * JW2
        ybase = s4 * SJW + t * 128
        joff = t * 128 + h * JW2

        # four 2048-idx gathers (ring-resident, 4-queue pipelined):
        # G[i, jl*128 + e], e = s*32 + r*8 + c
        G = gp.tile([128, JW2 * 128], F16, tag="G")
        in_ap = bass.AP(tabs[g].tensor,
                        tabs[g].offset + ybase * XT * 128,
                        [[128, 186 * XT], [1, 128]])
        NSUB = 2048
        for m in range(4):
            idxsl = bass.AP(Cw.tensor, Cw.offset + joff * 8 + m * (NSUB // 16),
                            [[Cw.ap[0][0], 128], [1, NSUB // 16]])
            q = cnt["q"] % 4
            cnt["q"] += 1
            nc.gpsimd.dma_gather(
                out_ap=bass.AP(G.tensor, G.offset + m * 16 * 128,
                               [[G.ap[0][0], 128], [128, 16], [1, 128]]),
                in_ap=in_ap,
                idxs_ap=idxsl,
                num_idxs=NSUB,
                num_idxs_reg=NSUB,
                elem_size=128,
                elem_step=128,
                single_packet=False,
                queue_num=q,
            )

        # combine: G[i, jl*128 + s*32 + r*8 + c] *= wp2[i, (s*4+r)*SJW + jb+jl]
        # (bcast over c) in two halves
        for m in range(2):
            src1 = bass.AP(wp2.tensor, wp2.offset + joff + m * 32,
                           [wp2.ap[0], [1, JW2 // 2], [4 * SJW, 4], [SJW, 4],
                            [0, 8]])
            src0 = bass.AP(G.tensor, G.offset + m * 32 * 128,
                           [G.ap[0], [128, JW2 // 2], [32, 4], [8, 4], [1, 8]])
            nc.vector.tensor_tensor(src0, src0, src1, op=OP.mult)

        def halve(buf, stride, n, tag, npx=JW2):
            o = lp.tile([128, npx * stride * (n // 2)], F16, tag=tag)
            i0 = bass.AP(buf.tensor, buf.offset,
                         [buf.ap[0], [stride * n, npx], [stride * 2, n // 2], [1, stride]])
            i1 = bass.AP(buf.tensor, buf.offset + stride,
                         [buf.ap[0], [stride * n, npx], [stride * 2, n // 2], [1, stride]])
            od = bass.AP(o.tensor, o.offset,
                         [o.ap[0], [stride * (n // 2), npx], [stride, n // 2], [1, stride]])
            nc.vector.tensor_tensor(od, i0, i1, op=OP.add)
            return o

        L1 = halve(G, 32, 4, "L1")
        L2 = halve(L1, 32, 2, "L2")
        L3 = halve(L2, 8, 4, "L3")
        of = outp.tile([128, 8 * JW2], F32, tag="of")
        i0 = bass.AP(L3.tensor, L3.offset, [L3.ap[0], [1, 8], [16, JW2]])
        i1 = bass.AP(L3.tensor, L3.offset + 8, [L3.ap[0], [1, 8], [16, JW2]])
        od = bass.AP(of.tensor, of.offset, [of.ap[0], [JW2, 8], [1, JW2]])
        nc.vector.tensor_tensor(od, i0, i1, op=OP.add)

        # blocked layout [RPC, W//64, C, 64]: 2KB contiguous per partition
        dsto = bass.AP(out, (IG * (W // JW2) + jb // JW2) * C * JW2,
                       [[(W // JW2) * C * JW2, 128], [1, C * JW2]])
        eng_out().dma_start(dsto, of[:])

    # ---------------- emission schedule --------------------------------
    # Serial phases: build both tables first (gather desc-gen on GpSimd
    # thrashes the SBUF ports DVE needs, so overlapping the build with the
    # gather phase slows the interleave copies ~5x). Supers are
    # software-pipelined one ahead so weights compute during gathers.
    for yb in range(N_YB):
        build_block(0, yb)
        build_block(1, yb)

    run_order = [(0, 0), (1, 0), (0, 1), (1, 1), (0, 2), (1, 2), (0, 3), (1, 3)]
    sups = {run_order[0]: super_tile(*run_order[0])}
    for i, (g, s4) in enumerate(run_order):
        if i + 1 < len(run_order):
            gn, sn = run_order[i + 1]
            sups[(gn, sn)] = super_tile(gn, sn)
        for t in range(4):
            for h in range(2):
                half_tile(g, s4, t, h, *sups[(g, s4)])
        del sups[(g, s4)]


_NC_CACHE = None


def kernel(x: np.ndarray, grid: np.ndarray) -> np.ndarray:
    global _NC_CACHE
    if _NC_CACHE is None:
        _NC_CACHE = build_nc()
    nc = _NC_CACHE

    x0 = np.ascontiguousarray(x[0], dtype=np.float32)        # [C, H, W]
    g0 = np.ascontiguousarray(grid[0], dtype=np.float32)     # [H, W, 2]

    in_maps = []
    for k in range(N_CORES):
        I0 = k * RPC
        xsl = np.zeros((C, YS + 4, XS), dtype=np.float16)
        c0 = I0 - PAD
        lo, hi = max(0, c0), min(W, c0 + XS)
        xsl[:, PAD:PAD + H, lo - c0:hi - c0] = x0[:, :, lo:hi].astype(np.float16)
        grc = np.ascontiguousarray(g0[I0:I0 + RPC]).copy()
        grc[..., 0] -= I0 / 1024.0   # fold per-core x-base into gx
        in_maps.append({"xs": xsl, "gr": grc})

    res = run_bass_kernel_spmd(nc, in_maps, core_ids=list(range(N_CORES)),
                               trace=False)
    global _LAST_EXEC_NS, _LAST_RES
    _LAST_EXEC_NS = res.exec_time_ns
    _LAST_RES = res
    out = np.empty((1, C, H, W), dtype=np.float32)
    for k in range(N_CORES):
        blk = res.results[k]["out"]          # [RPC, W//64, C, 64]
        out[0, :, k * RPC:(k + 1) * RPC, :] = (
            blk.transpose(2, 0, 1, 3).reshape(C, RPC, W))
    return out


# revision 13
# speedup vs baseline: 1.3014x; 1.0133x over previous
"""Bicubic grid_sample (transpose-like warp) for Trainium2, 8 NeuronCores.

Strategy: shard output rows across cores (256 rows/core). The warp maps
output (i, j) -> input (y ~ j +- 21, x ~ i +- 21), so each core needs an
x-column slab of the image. On device, repack the slab into a patch table
in DRAM where each 256B unit holds the full 4x4x8ch bicubic patch at
(y0, x0) (fp16). v3: slab ships as f16 (halves slab DMA), gathers merged
to one 8192-idx call per 64-col half-tile (4x fewer calls), weight
products built with fully-contiguous (s,r)-major ops (no strided ACT
copies), and row-group 1's table build is emitted interleaved with
row-group 0's gather/combine so the build streams through the latency
bubbles of the gather phase.
"""
import os, sys, types
sys.path.insert(0, "/opt/trn_rl_repo")
import numpy as np

try:  # register NTFF profile hook so BASS_TRACE=1 can measure HW time
    import antenv
    if "antenv.axon_hooks" not in sys.modules:
        from trn_agent_boot.trn_boot import _ntff_profile_via_ctypes
        _h = _ntff_profile_via_ctypes("/opt/axon/libaxon_pjrt.so")
        _m = types.ModuleType("antenv.axon_hooks")
        _m.get_axon_ntff_profile_hook = lambda: _h
        _m.set_axon_ntff_profile_hook = lambda h: None
        sys.modules["antenv.axon_hooks"] = _m
        antenv.axon_hooks = _m
except Exception:
    pass

import concourse.bass as bass
import concourse.bacc as bacc
import concourse.mybir as mybir
import concourse.tile as tile
from concourse import library_config
from concourse.bass_utils import run_bass_kernel_spmd

F32 = mybir.dt.float32
F16 = mybir.dt.float16
I16 = mybir.dt.int16
I32 = mybir.dt.int32
OP = mybir.AluOpType

N_CORES = 8
H = W = 2048
C = 8
RPC = H // N_CORES          # output rows per core = 256
PAD = 24                    # y halo rows on each side
YS = H + 2 * PAD            # 2096 slab rows
XS = 308                    # slab cols: [I0-24, I0+284)
XT = 176                    # table cols per row-group
XH = 88                     # x-half of the table staging buffer
YT = YS + 16                # table rows incl. pad so in_ap window stays in-bounds
SJW = 512                   # super-tile width (weights/idx granularity)
JW2 = 64                    # half-tile width (gather/combine granularity)
A = -0.75                   # bicubic constant
YB = 124                    # y-block rows for table build
N_YB = (YS + YB - 1) // YB  # 17


def build_nc():
    nc = bacc.Bacc("TRN2", target_bir_lowering=False, debug=False,
                   num_devices=N_CORES, num_swdge_queues=4)
    xs = nc.dram_tensor("xs", [C, YS + 4, XS], F16, kind="ExternalInput")
    gr = nc.dram_tensor("gr", [RPC, W, 2], F32, kind="ExternalInput")
    out = nc.dram_tensor("out", [RPC, W // JW2, C, JW2], F32,
                         kind="ExternalOutput")

    with tile.TileContext(nc) as tc:
        nc.gpsimd.load_library(library_config.mlp)
        import contextlib
        with contextlib.ExitStack() as ctx:
            _build_body(ctx, tc, nc, xs, gr, out)
    nc.compile()
    return nc


def _build_body(ctx, tc, nc, xs, gr, out):
    tabpool = ctx.enter_context(tc.tile_pool(name="tab", bufs=1, space="DRAM"))
    # phase-1 pools
    tpool = ctx.enter_context(tc.tile_pool(name="t", bufs=2))
    tg2pool = ctx.enter_context(tc.tile_pool(name="tg2", bufs=1))
    # phase-2 pools
    gridp = ctx.enter_context(tc.tile_pool(name="grid", bufs=2))
    wrk = ctx.enter_context(tc.tile_pool(name="wrk", bufs=1))
    wpp = ctx.enter_context(tc.tile_pool(name="wpp", bufs=2))
    idxp = ctx.enter_context(tc.tile_pool(name="idx", bufs=2))
    idxs1 = ctx.enter_context(tc.tile_pool(name="idx1", bufs=1))
    gp = ctx.enter_context(tc.tile_pool(name="g", bufs=2))
    lp = ctx.enter_context(tc.tile_pool(name="l", bufs=1))
    outp = ctx.enter_context(tc.tile_pool(name="out", bufs=2))

    tabs = []
    for g in range(2):
        tabg = tabpool.tile([YT * XT, 128], F16, tag=f"tab{g}")
        tabs.append(tabg)

    hwdge = [nc.sync, nc.scalar]
    cnt = {"dma": 0, "cp": 0, "q": 0}

    def eng():
        # phase-1 bulk traffic: alternate both HWDGE engines
        cnt["dma"] += 1
        return hwdge[cnt["dma"] % 2]

    def eng_prep():
        # phase-2 small DMAs go through SWDGE (gpsimd): the HWDGE queues are
        # backed up for ~1ms draining phase-1's table writes, and their
        # in-order FIFO would stall prep/output behind that drain.
        return nc.gpsimd

    def eng_out():
        return nc.gpsimd

    def ccopy(dst, src):
        cnt["cp"] += 1
        if cnt["cp"] % 2 == 0:
            nc.vector.tensor_copy(dst, src)
        else:
            nc.scalar.copy(dst, src)

    # ---------------- phase 1: repack xs -> table[g], one y-block ----------
    def build_block(g, yb):
        y0 = yb * YB
        rows = min(YB, YS - y0)
        # one DMA per row-shift r covering all 8 channels (slab is f16)
        t4 = tpool.tile([128, 4 * 8 * 179], F16, tag="xsb")
        for r in range(4):
            eng().dma_start(
                bass.AP(t4.tensor, t4.offset + r * 8 * 179,
                        [[t4.ap[0][0], rows], [1, 8 * 179]]),
                bass.AP(xs, (y0 + r) * XS + 128 * g,
                        [[XS, rows], [(YS + 4) * XS, 8], [1, 179]]))
        for h in range(2):
            tg2 = tg2pool.tile([128, XH * 128], F16, tag="tg2")
            for r in range(4):
                # merged interleave+shift:
                # tg2[p, xu*128 + s*32 + r*8 + c] = t[p, c*179 + XH*h + xu + s]
                dst = bass.AP(tg2.tensor, tg2.offset + r * 8,
                              [[tg2.ap[0][0], rows], [32, 4], [128, XH], [1, 8]])
                srcap = bass.AP(t4.tensor, t4.offset + r * 8 * 179 + XH * h,
                                [[t4.ap[0][0], rows], [1, 4], [1, XH], [179, 8]])
                ccopy(dst, srcap)
            # one HWDGE DMA, contiguous 22.5KB per row on both sides
            dsta = bass.AP(tabs[g].tensor,
                           tabs[g].offset + (y0 * XT + h * XH) * 128,
                           [[XT * 128, rows], [1, XH * 128]])
            eng().dma_start(dsta, tg2[:rows, :])

    # ---------------- phase 2: per super-tile weights+idx, gather+combine --
    def cubic(t, tag, outdt, opool):
        # returns w0..w3 tiles [128, SJW] in outdt; all-DVE (no cross-engine
        # handoffs); scratch tags shared between calls (sequential use).
        TS = nc.vector.tensor_scalar
        TT = nc.vector.tensor_tensor
        s0 = wrk.tile([128, SJW], F32, tag="c_s0")
        TS(s0[:], t[:], 1.0, None, op0=OP.add)
        w0f = wrk.tile([128, SJW], F32, tag="c_w0f")
        TS(w0f[:], s0[:], A, -5.0 * A, op0=OP.mult, op1=OP.add)
        TT(w0f[:], w0f[:], s0[:], op=OP.mult)
        TS(w0f[:], w0f[:], 8.0 * A, None, op0=OP.add)
        TT(w0f[:], w0f[:], s0[:], op=OP.mult)
        w0 = opool.tile([128, SJW], outdt, tag=f"w0{tag}")
        TS(w0[:], w0f[:], -4.0 * A, None, op0=OP.add)
        # w1
        w1f = wrk.tile([128, SJW], F32, tag="c_w1f")
        TS(w1f[:], t[:], A + 2.0, -(A + 3.0), op0=OP.mult, op1=OP.add)
        TT(w1f[:], w1f[:], t[:], op=OP.mult)
        TT(w1f[:], w1f[:], t[:], op=OP.mult)
        w1 = opool.tile([128, SJW], outdt, tag=f"w1{tag}")
        TS(w1[:], w1f[:], 1.0, None, op0=OP.add)
        # w2: u = 1 - t
        u = wrk.tile([128, SJW], F32, tag="c_u")
        TS(u[:], t[:], -1.0, 1.0, op0=OP.mult, op1=OP.add)
        w2f = wrk.tile([128, SJW], F32, tag="c_w2f")
        TS(w2f[:], u[:], A + 2.0, -(A + 3.0), op0=OP.mult, op1=OP.add)
        TT(w2f[:], w2f[:], u[:], op=OP.mult)
        TT(w2f[:], w2f[:], u[:], op=OP.mult)
        w2 = opool.tile([128, SJW], outdt, tag=f"w2{tag}")
        TS(w2[:], w2f[:], 1.0, None, op0=OP.add)
        # w3 = 1 - w0 - w1 - w2 (in f32 then cast)
        w3f = wrk.tile([128, SJW], F32, tag="c_w3f")
        TT(w3f[:], w0[:], w1[:], op=OP.add)
        TT(w3f[:], w3f[:], w2[:], op=OP.add)
        w3 = opool.tile([128, SJW], outdt, tag=f"w3{tag}")
        TS(w3[:], w3f[:], -1.0, 1.0, op0=OP.mult, op1=OP.add)
        return [w0, w1, w2, w3]

    def floorpair(v, tag):
        # vi/co scratch shared between calls; vf/fr persist per-dir
        vi = wrk.tile([128, SJW], I32, tag="f_vi")
        nc.vector.tensor_copy(vi[:], v[:])
        vf = wrk.tile([128, SJW], F32, tag=f"vf{tag}")
        nc.vector.tensor_copy(vf[:], vi[:])
        co = wrk.tile([128, SJW], F32, tag="f_co")
        nc.vector.tensor_tensor(co[:], vf[:], v[:], op=OP.is_gt)
        nc.vector.tensor_tensor(vf[:], vf[:], co[:], op=OP.subtract)
        fr = wrk.tile([128, SJW], F32, tag=f"fr{tag}")
        nc.vector.tensor_tensor(fr[:], v[:], vf[:], op=OP.subtract)
        return vf, fr

    def super_tile(g, s4):
        """Weights + wrapped idx for 512 output cols of row-group g.
        Returns (wp2, Cw): wp2[i, (s*4+r)*SJW + j] = wx_s(i,j)*wy_r(i,j)."""
        IG = g * 128
        jb4 = s4 * SJW
        gt = gridp.tile([128, SJW * 2], F32, tag="gt")
        nc.gpsimd.dma_start(
            gt[:],
            bass.AP(gr, IG * W * 2 + jb4 * 2, [[W * 2, 128], [1, SJW * 2]]))
        gx = bass.AP(gt.tensor, gt.offset, [gt.ap[0], [2, SJW]])
        gy = bass.AP(gt.tensor, gt.offset + 1, [gt.ap[0], [2, SJW]])

        lx = wrk.tile([128, SJW], F32, tag="lx")
        ly = wrk.tile([128, SJW], F32, tag="ly")
        nc.vector.tensor_scalar(lx[:], gx, 1024.0, 1047.5 - IG,
                                op0=OP.mult, op1=OP.add)
        nc.vector.tensor_scalar(ly[:], gy, 1024.0, 1046.5 - jb4,
                                op0=OP.mult, op1=OP.add)
        fx, tx = floorpair(lx, "x")
        fy, ty = floorpair(ly, "y")

        # idxf = fy*XT + fx - 1 (f32, exact)
        idxf = wrk.tile([128, SJW], F32, tag="idxf")
        nc.vector.scalar_tensor_tensor(idxf[:], fy[:], float(XT), fx[:],
                                       op0=OP.mult, op1=OP.add)
        # per-sub-tile rebase to the 186-row gather window, cast to i16
        idx16 = idxs1.tile([128, SJW], I16, tag="idx16")
        for t in range(SJW // 128):
            nc.vector.tensor_scalar(
                bass.AP(idx16.tensor, idx16.offset + t * 128,
                        [[idx16.ap[0][0], 128], [1, 128]]),
                bass.AP(idxf.tensor, idxf.offset + t * 128,
                        [[idxf.ap[0][0], 128], [1, 128]]),
                -1.0 - t * 128.0 * XT, None, op0=OP.add)

        # deferred steps (emitted interleaved with the previous supertile's
        # half-tiles so pool-queue waits resolve before issue):
        D = idxs1.tile([128, 8 * SJW], I16, tag="D")
        Cw = idxp.tile([128, 8 * SJW], I16, tag="Cw")

        def do_folds():
            # fold [128, SJW] -> wrapped [16, 8*SJW]: D[p, kS+j] = idx16[16k+p, j]
            for k in range(8):
                src = bass.AP(idx16.tensor,
                              idx16.offset + 16 * k * idx16.ap[0][0],
                              [[idx16.ap[0][0], 16], [1, SJW]])
                dst = bass.AP(D.tensor, D.offset + k * SJW,
                              [[D.ap[0][0], 16], [1, SJW]])
                eng_prep().dma_start(dst, src)

        def do_repl():
            # replicate D to all 8 gpsimd core bands (7 independent copies)
            for rep in range(1, 8):
                src = bass.AP(D.tensor, D.offset, [[D.ap[0][0], 16], [1, 8 * SJW]])
                dst = bass.AP(D.tensor, D.offset + 16 * rep * D.ap[0][0],
                              [[D.ap[0][0], 16], [1, 8 * SJW]])
                eng_prep().dma_start(dst, src)

        def do_cwint():
            # full-width interleave: Cw[p, 8j+k] = D[p%16-band, k*SJW+j]
            nc.vector.tensor_copy(
                bass.AP(Cw.tensor, Cw.offset, [[Cw.ap[0][0], 128], [8, SJW], [1, 8]]),
                bass.AP(D.tensor, D.offset, [[D.ap[0][0], 128], [1, SJW], [SJW, 8]]))

        wx = cubic(tx, "x", F16, wrk)
        wy = cubic(ty, "y", F16, wrk)
        # wp2[i, (s*4+r)*SJW + j]: 16 fully-contiguous DVE multiplies
        wp2 = wpp.tile([128, 16 * SJW], F16, tag="wp2")
        for s in range(4):
            for r in range(4):
                dst = bass.AP(wp2.tensor, wp2.offset + (s * 4 + r) * SJW,
                              [wp2.ap[0], [1, SJW]])
                nc.vector.tensor_tensor(dst, wx[s][:], wy[r][:], op=OP.mult)
        return wp2, Cw, [do_folds, do_repl, do_cwint]

    def half_tile(g, s4, t, h, wp2, Cw):
        IG = g * 128
        jb = s4 * SJW + t * 128 + h * JW2
        ybase = s4 * SJW + t * 128
        joff = t * 128 + h * JW2

        # four 2048-idx gathers (ring-resident, 4-queue pipelined):
        # G[i, jl*128 + e], e = s*32 + r*8 + c
        G = gp.tile([128, JW2 * 128], F16, tag="G")
        in_ap = bass.AP(tabs[g].tensor,
                        tabs[g].offset + ybase * XT * 128,
                        [[128, 186 * XT], [1, 128]])
        NSUB = 2048
        for m in range(4):
            idxsl = bass.AP(Cw.tensor, Cw.offset + joff * 8 + m * (NSUB // 16),
                            [[Cw.ap[0][0], 128], [1, NSUB // 16]])
            q = cnt["q"] % 4
            cnt["q"] += 1
            nc.gpsimd.dma_gather(
                out_ap=bass.AP(G.tensor, G.offset + m * 16 * 128,
                               [[G.ap[0][0], 128], [128, 16], [1, 128]]),
                in_ap=in_ap,
                idxs_ap=idxsl,
                num_idxs=NSUB,
                num_idxs_reg=NSUB,
                elem_size=128,
                elem_step=128,
                single_packet=False,
                queue_num=q,
            )

        # combine: G[i, jl*128 + s*32 + r*8 + c] *= wp2[i, (s*4+r)*SJW + jb+jl]
        # (bcast over c) in two halves
        for m in range(2):
            src1 = bass.AP(wp2.tensor, wp2.offset + joff + m * 32,
                           [wp2.ap[0], [1, JW2 // 2], [4 * SJW, 4], [SJW, 4],
                            [0, 8]])
            src0 = bass.AP(G.tensor, G.offset + m * 32 * 128,
                           [G.ap[0], [128, JW2 // 2], [32, 4], [8, 4], [1, 8]])
            nc.vector.tensor_tensor(src0, src0, src1, op=OP.mult)

        def halve(buf, stride, n, tag, npx=JW2):
            o = lp.tile([128, npx * stride * (n // 2)], F16, tag=tag)
            i0 = bass.AP(buf.tensor, buf.offset,
                         [buf.ap[0], [stride * n, npx], [stride * 2, n // 2], [1, stride]])
            i1 = bass.AP(buf.tensor, buf.offset + stride,
                         [buf.ap[0], [stride * n, npx], [stride * 2, n // 2], [1, stride]])
            od = bass.AP(o.tensor, o.offset,
                         [o.ap[0], [stride * (n // 2), npx], [stride, n // 2], [1, stride]])
            nc.vector.tensor_tensor(od, i0, i1, op=OP.add)
            return o

        L1 = halve(G, 32, 4, "L1")
        L2 = halve(L1, 32, 2, "L2")
        L3 = halve(L2, 8, 4, "L3")
        of = outp.tile([128, 8 * JW2], F32, tag="of")
        i0 = bass.AP(L3.tensor, L3.offset, [L3.ap[0], [1, 8], [16, JW2]])
        i1 = bass.AP(L3.tensor, L3.offset + 8, [L3.ap[0], [1, 8], [16, JW2]])
        od = bass.AP(of.tensor, of.offset, [of.ap[0], [JW2, 8], [1, JW2]])
        nc.vector.tensor_tensor(od, i0, i1, op=OP.add)

        # blocked layout [RPC, W//64, C, 64]: 2KB contiguous per partition
        dsto = bass.AP(out, (IG * (W // JW2) + jb // JW2) * C * JW2,
                       [[(W // JW2) * C * JW2, 128], [1, C * JW2]])
        eng_out().dma_start(dsto, of[:])

    # ---------------- emission schedule --------------------------------
    # Serial phases: build both tables first (gather desc-gen on GpSimd
    # thrashes the SBUF ports DVE needs, so overlapping the build with the
    # gather phase slows the interleave copies ~5x). Supers are
    # software-pipelined one ahead so weights compute during gathers.
    for yb in range(N_YB):
        build_block(0, yb)
        build_block(1, yb)

    run_order = [(0, 0), (1, 0), (0, 1), (1, 1), (0, 2), (1, 2), (0, 3), (1, 3)]
    *s0head, s0steps = super_tile(*run_order[0])
    for st in s0steps:  # first supertile: run deferred steps immediately
        st()
    sups = {run_order[0]: s0head}
    for i, (g, s4) in enumerate(run_order):
        steps = []
        if i + 1 < len(run_order):
            gn, sn = run_order[i + 1]
            *head, steps = super_tile(gn, sn)
            sups[(gn, sn)] = head
        # interleave next supertile's idx-DMA chain between this one's
        # half-tiles: each link's producers finish well before the in-order
        # pool queue reaches it, so nothing blocks the gather stream
        slots = {2: 0, 4: 1, 5: 2}  # after (t,h) pairs 1,2 -> folds, repl, cwint
        k = 0
        for t in range(4):
            for h in range(2):
                half_tile(g, s4, t, h, *sups[(g, s4)])
                k += 1
                if k in slots and steps:
                    steps[slots[k]]()
        del sups[(g, s4)]


_NC_CACHE = None


def kernel(x: np.ndarray, grid: np.ndarray) -> np.ndarray:
    global _NC_CACHE
    if _NC_CACHE is None:
        _NC_CACHE = build_nc()
    nc = _NC_CACHE

    x0 = np.ascontiguousarray(x[0], dtype=np.float32)        # [C, H, W]
    g0 = np.ascontiguousarray(grid[0], dtype=np.float32)     # [H, W, 2]

    in_maps = []
    for k in range(N_CORES):
        I0 = k * RPC
        xsl = np.zeros((C, YS + 4, XS), dtype=np.float16)
        c0 = I0 - PAD
        lo, hi = max(0, c0), min(W, c0 + XS)
        xsl[:, PAD:PAD + H, lo - c0:hi - c0] = x0[:, :, lo:hi].astype(np.float16)
        grc = np.ascontiguousarray(g0[I0:I0 + RPC]).copy()
        grc[..., 0] -= I0 / 1024.0   # fold per-core x-base into gx
        in_maps.append({"xs": xsl, "gr": grc})

    res = run_bass_kernel_spmd(nc, in_maps, core_ids=list(range(N_CORES)),
                               trace=False)
    global _LAST_EXEC_NS, _LAST_RES
    _LAST_EXEC_NS = res.exec_time_ns
    _LAST_RES = res
    out = np.empty((1, C, H, W), dtype=np.float32)
    for k in range(N_CORES):
        blk = res.results[k]["out"]          # [RPC, W//64, C, 64]
        out[0, :, k * RPC:(k + 1) * RPC, :] = (
            blk.transpose(2, 0, 1, 3).reshape(C, RPC, W))
    return out


# revision 16
# speedup vs baseline: 1.3098x; 1.0064x over previous
"""Bicubic grid_sample (transpose-like warp) for Trainium2, 8 NeuronCores.

Strategy: shard output rows across cores (256 rows/core). The warp maps
output (i, j) -> input (y ~ j +- 21, x ~ i +- 21), so each core needs an
x-column slab of the image. On device, repack the slab into a patch table
in DRAM where each 256B unit holds the full 4x4x8ch bicubic patch at
(y0, x0) (fp16). v8: slab ships as f16 (halves slab DMA), weight
products built with fully-contiguous (s,r)-major DVE ops (no strided ACT
copies), phase-2 small DMAs issued via SWDGE (async rings; the HWDGE
FIFOs are busy draining phase-1's table writes), and the idx fold/
replicate/interleave chain for supertile k+1 is emitted between
supertile k's half-tiles so each link's producers finish before the
in-order pool queue reaches it (removes ~190us gather stalls at
supertile boundaries). Gathers stay 2048-idx x 4 queues: bigger calls
overflow the SWDGE descriptor rings and serialize desc-gen with drain.
Measured wall: the 256B-record random gather is SDMA per-descriptor
overhead-bound (~52ns fixed + ~0.1ns/B per engine); SBUF-source
transposed gather is worse (2B-per-partition xbar writes, ~30GB/s).
"""
import os, sys, types
sys.path.insert(0, "/opt/trn_rl_repo")
import numpy as np

try:  # register NTFF profile hook so BASS_TRACE=1 can measure HW time
    import antenv
    if "antenv.axon_hooks" not in sys.modules:
        from trn_agent_boot.trn_boot import _ntff_profile_via_ctypes
        _h = _ntff_profile_via_ctypes("/opt/axon/libaxon_pjrt.so")
        _m = types.ModuleType("antenv.axon_hooks")
        _m.get_axon_ntff_profile_hook = lambda: _h
        _m.set_axon_ntff_profile_hook = lambda h: None
        sys.modules["antenv.axon_hooks"] = _m
        antenv.axon_hooks = _m
except Exception:
    pass

import concourse.bass as bass
import concourse.bacc as bacc
import concourse.mybir as mybir
import concourse.tile as tile
from concourse import library_config
from concourse.bass_utils import run_bass_kernel_spmd

F32 = mybir.dt.float32
F16 = mybir.dt.float16
I16 = mybir.dt.int16
I32 = mybir.dt.int32
OP = mybir.AluOpType

N_CORES = 8
H = W = 2048
C = 8
RPC = H // N_CORES          # output rows per core = 256
PAD = 24                    # y halo rows on each side
YS = H + 2 * PAD            # 2096 slab rows
XS = 308                    # slab cols: [I0-24, I0+284)
XT = 176                    # table cols per row-group
XH = 88                     # x-half of the table staging buffer
YT = YS + 16                # table rows incl. pad so in_ap window stays in-bounds
SJW = 512                   # super-tile width (weights/idx granularity)
JW2 = 64                    # half-tile width (gather/combine granularity)
A = -0.75                   # bicubic constant
YB = 124                    # y-block rows for table build
N_YB = (YS + YB - 1) // YB  # 17


def build_nc():
    nc = bacc.Bacc("TRN2", target_bir_lowering=False, debug=False,
                   num_devices=N_CORES, num_swdge_queues=4)
    xs = nc.dram_tensor("xs", [C, YS + 4, XS], F16, kind="ExternalInput")
    gr = nc.dram_tensor("gr", [RPC, W, 2], F32, kind="ExternalInput")
    out = nc.dram_tensor("out", [RPC, W // JW2, C, JW2], F32,
                         kind="ExternalOutput")

    with tile.TileContext(nc) as tc:
        nc.gpsimd.load_library(library_config.mlp)
        import contextlib
        with contextlib.ExitStack() as ctx:
            _build_body(ctx, tc, nc, xs, gr, out)
    nc.compile()
    return nc


def _build_body(ctx, tc, nc, xs, gr, out):
    tabpool = ctx.enter_context(tc.tile_pool(name="tab", bufs=1, space="DRAM"))
    # phase-1 pools
    tpool = ctx.enter_context(tc.tile_pool(name="t", bufs=2))
    tg2pool = ctx.enter_context(tc.tile_pool(name="tg2", bufs=1))
    # phase-2 pools
    gridp = ctx.enter_context(tc.tile_pool(name="grid", bufs=2))
    wrk = ctx.enter_context(tc.tile_pool(name="wrk", bufs=1))
    wpp = ctx.enter_context(tc.tile_pool(name="wpp", bufs=2))
    idxp = ctx.enter_context(tc.tile_pool(name="idx", bufs=2))
    idxs1 = ctx.enter_context(tc.tile_pool(name="idx1", bufs=1))
    gp = ctx.enter_context(tc.tile_pool(name="g", bufs=2))
    lp = ctx.enter_context(tc.tile_pool(name="l", bufs=1))
    outp = ctx.enter_context(tc.tile_pool(name="out", bufs=2))

    tabs = []
    for g in range(2):
        tabg = tabpool.tile([YT * XT, 128], F16, tag=f"tab{g}")
        tabs.append(tabg)

    hwdge = [nc.sync, nc.scalar]
    cnt = {"dma": 0, "cp": 0, "q": 0}

    def eng():
        # phase-1 bulk traffic: alternate both HWDGE engines
        cnt["dma"] += 1
        return hwdge[cnt["dma"] % 2]

    def eng_prep():
        # phase-2 small DMAs go through SWDGE (gpsimd): the HWDGE queues are
        # backed up for ~1ms draining phase-1's table writes, and their
        # in-order FIFO would stall prep/output behind that drain.
        return nc.gpsimd

    def eng_out():
        return nc.gpsimd

    def ccopy(dst, src):
        cnt["cp"] += 1
        if cnt["cp"] % 2 == 0:
            nc.vector.tensor_copy(dst, src)
        else:
            nc.scalar.copy(dst, src)

    # ---------------- phase 1: repack xs -> table[g], one y-block ----------
    def build_block(g, yb):
        y0 = yb * YB
        rows = min(YB, YS - y0)
        # one DMA per row-shift r covering all 8 channels (slab is f16)
        t4 = tpool.tile([128, 4 * 8 * 179], F16, tag="xsb")
        for r in range(4):
            eng().dma_start(
                bass.AP(t4.tensor, t4.offset + r * 8 * 179,
                        [[t4.ap[0][0], rows], [1, 8 * 179]]),
                bass.AP(xs, (y0 + r) * XS + 128 * g,
                        [[XS, rows], [(YS + 4) * XS, 8], [1, 179]]))
        for h in range(2):
            tg2 = tg2pool.tile([128, XH * 128], F16, tag="tg2")
            for r in range(4):
                # merged interleave+shift:
                # tg2[p, xu*128 + s*32 + r*8 + c] = t[p, c*179 + XH*h + xu + s]
                dst = bass.AP(tg2.tensor, tg2.offset + r * 8,
                              [[tg2.ap[0][0], rows], [32, 4], [128, XH], [1, 8]])
                srcap = bass.AP(t4.tensor, t4.offset + r * 8 * 179 + XH * h,
                                [[t4.ap[0][0], rows], [1, 4], [1, XH], [179, 8]])
                ccopy(dst, srcap)
            # one HWDGE DMA, contiguous 22.5KB per row on both sides
            dsta = bass.AP(tabs[g].tensor,
                           tabs[g].offset + (y0 * XT + h * XH) * 128,
                           [[XT * 128, rows], [1, XH * 128]])
            eng().dma_start(dsta, tg2[:rows, :])

    # ---------------- phase 2: per super-tile weights+idx, gather+combine --
    def cubic(t, tag, outdt, opool):
        # returns w0..w3 tiles [128, SJW] in outdt; all-DVE (no cross-engine
        # handoffs); scratch tags shared between calls (sequential use).
        TS = nc.vector.tensor_scalar
        TT = nc.vector.tensor_tensor
        s0 = wrk.tile([128, SJW], F32, tag="c_s0")
        TS(s0[:], t[:], 1.0, None, op0=OP.add)
        w0f = wrk.tile([128, SJW], F32, tag="c_w0f")
        TS(w0f[:], s0[:], A, -5.0 * A, op0=OP.mult, op1=OP.add)
        TT(w0f[:], w0f[:], s0[:], op=OP.mult)
        TS(w0f[:], w0f[:], 8.0 * A, None, op0=OP.add)
        TT(w0f[:], w0f[:], s0[:], op=OP.mult)
        w0 = opool.tile([128, SJW], outdt, tag=f"w0{tag}")
        TS(w0[:], w0f[:], -4.0 * A, None, op0=OP.add)
        # w1
        w1f = wrk.tile([128, SJW], F32, tag="c_w1f")
        TS(w1f[:], t[:], A + 2.0, -(A + 3.0), op0=OP.mult, op1=OP.add)
        TT(w1f[:], w1f[:], t[:], op=OP.mult)
        TT(w1f[:], w1f[:], t[:], op=OP.mult)
        w1 = opool.tile([128, SJW], outdt, tag=f"w1{tag}")
        TS(w1[:], w1f[:], 1.0, None, op0=OP.add)
        # w2: u = 1 - t
        u = wrk.tile([128, SJW], F32, tag="c_u")
        TS(u[:], t[:], -1.0, 1.0, op0=OP.mult, op1=OP.add)
        w2f = wrk.tile([128, SJW], F32, tag="c_w2f")
        TS(w2f[:], u[:], A + 2.0, -(A + 3.0), op0=OP.mult, op1=OP.add)
        TT(w2f[:], w2f[:], u[:], op=OP.mult)
        TT(w2f[:], w2f[:], u[:], op=OP.mult)
        w2 = opool.tile([128, SJW], outdt, tag=f"w2{tag}")
        TS(w2[:], w2f[:], 1.0, None, op0=OP.add)
        # w3 = 1 - w0 - w1 - w2 (in f32 then cast)
        w3f = wrk.tile([128, SJW], F32, tag="c_w3f")
        TT(w3f[:], w0[:], w1[:], op=OP.add)
        TT(w3f[:], w3f[:], w2[:], op=OP.add)
        w3 = opool.tile([128, SJW], outdt, tag=f"w3{tag}")
        TS(w3[:], w3f[:], -1.0, 1.0, op0=OP.mult, op1=OP.add)
        return [w0, w1, w2, w3]

    def floorpair(v, tag):
        # vi/co scratch shared between calls; vf/fr persist per-dir
        vi = wrk.tile([128, SJW], I32, tag="f_vi")
        nc.vector.tensor_copy(vi[:], v[:])
        vf = wrk.tile([128, SJW], F32, tag=f"vf{tag}")
        nc.vector.tensor_copy(vf[:], vi[:])
        co = wrk.tile([128, SJW], F32, tag="f_co")
        nc.vector.tensor_tensor(co[:], vf[:], v[:], op=OP.is_gt)
        nc.vector.tensor_tensor(vf[:], vf[:], co[:], op=OP.subtract)
        fr = wrk.tile([128, SJW], F32, tag=f"fr{tag}")
        nc.vector.tensor_tensor(fr[:], v[:], vf[:], op=OP.subtract)
        return vf, fr

    def super_tile(g, s4):
        """Weights + wrapped idx for 512 output cols of row-group g.
        Returns (wp2, Cw): wp2[i, (s*4+r)*SJW + j] = wx_s(i,j)*wy_r(i,j)."""
        IG = g * 128
        jb4 = s4 * SJW
        gt = gridp.tile([128, SJW * 2], F32, tag="gt")
        nc.gpsimd.dma_start(
            gt[:],
            bass.AP(gr, IG * W * 2 + jb4 * 2, [[W * 2, 128], [1, SJW * 2]]))
        gx = bass.AP(gt.tensor, gt.offset, [gt.ap[0], [2, SJW]])
        gy = bass.AP(gt.tensor, gt.offset + 1, [gt.ap[0], [2, SJW]])

        lx = wrk.tile([128, SJW], F32, tag="lx")
        ly = wrk.tile([128, SJW], F32, tag="ly")
        nc.vector.tensor_scalar(lx[:], gx, 1024.0, 1047.5 - IG,
                                op0=OP.mult, op1=OP.add)
        nc.vector.tensor_scalar(ly[:], gy, 1024.0, 1046.5 - jb4,
                                op0=OP.mult, op1=OP.add)
        fx, tx = floorpair(lx, "x")
        fy, ty = floorpair(ly, "y")

        # idxf = fy*XT + fx - 1 (f32, exact)
        idxf = wrk.tile([128, SJW], F32, tag="idxf")
        nc.vector.scalar_tensor_tensor(idxf[:], fy[:], float(XT), fx[:],
                                       op0=OP.mult, op1=OP.add)
        # per-sub-tile rebase to the 186-row gather window, cast to i16
        idx16 = idxs1.tile([128, SJW], I16, tag="idx16")
        for t in range(SJW // 128):
            nc.vector.tensor_scalar(
                bass.AP(idx16.tensor, idx16.offset + t * 128,
                        [[idx16.ap[0][0], 128], [1, 128]]),
                bass.AP(idxf.tensor, idxf.offset + t * 128,
                        [[idxf.ap[0][0], 128], [1, 128]]),
                -1.0 - t * 128.0 * XT, None, op0=OP.add)

        # deferred steps (emitted interleaved with the previous supertile's
        # half-tiles so pool-queue waits resolve before issue):
        D = idxs1.tile([128, 8 * SJW], I16, tag="D")
        Cw = idxp.tile([128, 8 * SJW], I16, tag="Cw")

        def do_folds():
            # fold [128, SJW] -> wrapped [16, 8*SJW]: D[p, kS+j] = idx16[16k+p, j]
            for k in range(8):
                src = bass.AP(idx16.tensor,
                              idx16.offset + 16 * k * idx16.ap[0][0],
                              [[idx16.ap[0][0], 16], [1, SJW]])
                dst = bass.AP(D.tensor, D.offset + k * SJW,
                              [[D.ap[0][0], 16], [1, SJW]])
                eng_prep().dma_start(dst, src)

        def do_repl():
            # replicate D to all 8 gpsimd core bands (7 independent copies)
            for rep in range(1, 8):
                src = bass.AP(D.tensor, D.offset, [[D.ap[0][0], 16], [1, 8 * SJW]])
                dst = bass.AP(D.tensor, D.offset + 16 * rep * D.ap[0][0],
                              [[D.ap[0][0], 16], [1, 8 * SJW]])
                eng_prep().dma_start(dst, src)

        def do_cwint():
            # full-width interleave: Cw[p, 8j+k] = D[p%16-band, k*SJW+j]
            nc.vector.tensor_copy(
                bass.AP(Cw.tensor, Cw.offset, [[Cw.ap[0][0], 128], [8, SJW], [1, 8]]),
                bass.AP(D.tensor, D.offset, [[D.ap[0][0], 128], [1, SJW], [SJW, 8]]))

        wx = cubic(tx, "x", F16, wrk)
        wy = cubic(ty, "y", F16, wrk)
        # wp2[i, (s*4+r)*SJW + j]: 16 fully-contiguous DVE multiplies
        wp2 = wpp.tile([128, 16 * SJW], F16, tag="wp2")
        for s in range(4):
            for r in range(4):
                dst = bass.AP(wp2.tensor, wp2.offset + (s * 4 + r) * SJW,
                              [wp2.ap[0], [1, SJW]])
                nc.vector.tensor_tensor(dst, wx[s][:], wy[r][:], op=OP.mult)
        return wp2, Cw, [do_folds, do_repl, do_cwint]

    def half_tile(g, s4, t, h, wp2, Cw):
        IG = g * 128
        jb = s4 * SJW + t * 128 + h * JW2
        ybase = s4 * SJW + t * 128
        joff = t * 128 + h * JW2

        # four 2048-idx gathers (ring-resident, 4-queue pipelined):
        # G[i, jl*128 + e], e = s*32 + r*8 + c
        G = gp.tile([128, JW2 * 128], F16, tag="G")
        in_ap = bass.AP(tabs[g].tensor,
                        tabs[g].offset + ybase * XT * 128,
                        [[128, 186 * XT], [1, 128]])
        NSUB = 2048
        for m in range(4):
            idxsl = bass.AP(Cw.tensor, Cw.offset + joff * 8 + m * (NSUB // 16),
                            [[Cw.ap[0][0], 128], [1, NSUB // 16]])
            q = cnt["q"] % 4
            cnt["q"] += 1
            nc.gpsimd.dma_gather(
                out_ap=bass.AP(G.tensor, G.offset + m * 16 * 128,
                               [[G.ap[0][0], 128], [128, 16], [1, 128]]),
                in_ap=in_ap,
                idxs_ap=idxsl,
                num_idxs=NSUB,
                num_idxs_reg=NSUB,
                elem_size=128,
                elem_step=128,
                single_packet=False,
                queue_num=q,
            )

        # combine: G[i, jl*128 + s*32 + r*8 + c] *= wp2[i, (s*4+r)*SJW + jb+jl]
        # (bcast over c) in two halves
        for m in range(2):
            src1 = bass.AP(wp2.tensor, wp2.offset + joff + m * 32,
                           [wp2.ap[0], [1, JW2 // 2], [4 * SJW, 4], [SJW, 4],
                            [0, 8]])
            src0 = bass.AP(G.tensor, G.offset + m * 32 * 128,
                           [G.ap[0], [128, JW2 // 2], [32, 4], [8, 4], [1, 8]])
            nc.vector.tensor_tensor(src0, src0, src1, op=OP.mult)

        def halve(buf, stride, n, tag, npx=JW2):
            o = lp.tile([128, npx * stride * (n // 2)], F16, tag=tag)
            i0 = bass.AP(buf.tensor, buf.offset,
                         [buf.ap[0], [stride * n, npx], [stride * 2, n // 2], [1, stride]])
            i1 = bass.AP(buf.tensor, buf.offset + stride,
                         [buf.ap[0], [stride * n, npx], [stride * 2, n // 2], [1, stride]])
            od = bass.AP(o.tensor, o.offset,
                         [o.ap[0], [stride * (n // 2), npx], [stride, n // 2], [1, stride]])
            nc.vector.tensor_tensor(od, i0, i1, op=OP.add)
            return o

        L1 = halve(G, 32, 4, "L1")
        L2 = halve(L1, 32, 2, "L2")
        L3 = halve(L2, 8, 4, "L3")
        of = outp.tile([128, 8 * JW2], F32, tag="of")
        i0 = bass.AP(L3.tensor, L3.offset, [L3.ap[0], [1, 8], [16, JW2]])
        i1 = bass.AP(L3.tensor, L3.offset + 8, [L3.ap[0], [1, 8], [16, JW2]])
        od = bass.AP(of.tensor, of.offset, [of.ap[0], [JW2, 8], [1, JW2]])
        nc.vector.tensor_tensor(od, i0, i1, op=OP.add)

        # blocked layout [RPC, W//64, C, 64]: 2KB contiguous per partition
        dsto = bass.AP(out, (IG * (W // JW2) + jb // JW2) * C * JW2,
                       [[(W // JW2) * C * JW2, 128], [1, C * JW2]])
        eng_out().dma_start(dsto, of[:])

    # ---------------- emission schedule --------------------------------
    # Emit the first two supertiles' prep BEFORE phase 1 so their weights/
    # idx/Cw DVE work runs ahead of phase-1's interleave-copy backlog in the
    # in-order DVE queue (their pool-side DMAs complete during the build).
    # g-major run order + tab[0]-first build lets g=0 gathers start as soon
    # as tab[0]'s writes drain while tab[1]'s drain underneath them.
    run_order = [(0, 0), (0, 1), (0, 2), (0, 3), (1, 0), (1, 1), (1, 2), (1, 3)]
    sups = {}
    for key in run_order[:2]:
        *head, steps = super_tile(*key)
        for st in steps:
            st()
        sups[key] = head

    for yb in range(N_YB):
        build_block(0, yb)
    for yb in range(N_YB):
        build_block(1, yb)

    for i, (g, s4) in enumerate(run_order):
        steps = []
        if i + 1 < len(run_order) and run_order[i + 1] not in sups:
            key = run_order[i + 1]
            *head, steps = super_tile(key[0], key[1])
            sups[key] = head
        # interleave the next supertile's idx-DMA chain between this one's
        # half-tiles: each link's producers finish well before the in-order
        # pool queue reaches it, so nothing blocks the gather stream
        slots = {2: 0, 4: 1, 5: 2}  # after (t,h) pairs 1,2 -> folds, repl, cwint
        k = 0
        for t in range(4):
            for h in range(2):
                half_tile(g, s4, t, h, *sups[(g, s4)])
                k += 1
                if k in slots and steps:
                    steps[slots[k]]()
        del sups[(g, s4)]


_NC_CACHE = None


def kernel(x: np.ndarray, grid: np.ndarray) -> np.ndarray:
    global _NC_CACHE
    if _NC_CACHE is None:
        _NC_CACHE = build_nc()
    nc = _NC_CACHE

    x0 = np.ascontiguousarray(x[0], dtype=np.float32)        # [C, H, W]
    g0 = np.ascontiguousarray(grid[0], dtype=np.float32)     # [H, W, 2]

    in_maps = []
    for k in range(N_CORES):
        I0 = k * RPC
        xsl = np.zeros((C, YS + 4, XS), dtype=np.float16)
        c0 = I0 - PAD
        lo, hi = max(0, c0), min(W, c0 + XS)
        xsl[:, PAD:PAD + H, lo - c0:hi - c0] = x0[:, :, lo:hi].astype(np.float16)
        grc = np.ascontiguousarray(g0[I0:I0 + RPC]).copy()
        grc[..., 0] -= I0 / 1024.0   # fold per-core x-base into gx
        in_maps.append({"xs": xsl, "gr": grc})

    res = run_bass_kernel_spmd(nc, in_maps, core_ids=list(range(N_CORES)),
                               trace=False)
    global _LAST_EXEC_NS, _LAST_RES
    _LAST_EXEC_NS = res.exec_time_ns
    _LAST_RES = res
    out = np.empty((1, C, H, W), dtype=np.float32)
    for k in range(N_CORES):
        blk = res.results[k]["out"]          # [RPC, W//64, C, 64]
        out[0, :, k * RPC:(k + 1) * RPC, :] = (
            blk.transpose(2, 0, 1, 3).reshape(C, RPC, W))
    return out


# revision 17
# speedup vs baseline: 1.3162x; 1.0049x over previous
"""Bicubic grid_sample (transpose-like warp) for Trainium2, 8 NeuronCores.

Strategy: shard output rows across cores (256 rows/core). The warp maps
output (i, j) -> input (y ~ j +- 21, x ~ i +- 21), so each core needs an
x-column slab of the image. On device, repack the slab into a patch table
in DRAM where each 256B unit holds the full 4x4x8ch bicubic patch at
(y0, x0) (fp16). v8: slab ships as f16 (halves slab DMA), weight
products built with fully-contiguous (s,r)-major DVE ops (no strided ACT
copies), phase-2 small DMAs issued via SWDGE (async rings; the HWDGE
FIFOs are busy draining phase-1's table writes), and the idx fold/
replicate/interleave chain for supertile k+1 is emitted between
supertile k's half-tiles so each link's producers finish before the
in-order pool queue reaches it (removes ~190us gather stalls at
supertile boundaries). Gathers stay 2048-idx x 4 queues: bigger calls
overflow the SWDGE descriptor rings and serialize desc-gen with drain.
Measured wall: the 256B-record random gather is SDMA per-descriptor
overhead-bound (~52ns fixed + ~0.1ns/B per engine); SBUF-source
transposed gather is worse (2B-per-partition xbar writes, ~30GB/s).
"""
import os, sys, types
sys.path.insert(0, "/opt/trn_rl_repo")
import numpy as np

try:  # register NTFF profile hook so BASS_TRACE=1 can measure HW time
    import antenv
    if "antenv.axon_hooks" not in sys.modules:
        from trn_agent_boot.trn_boot import _ntff_profile_via_ctypes
        _h = _ntff_profile_via_ctypes("/opt/axon/libaxon_pjrt.so")
        _m = types.ModuleType("antenv.axon_hooks")
        _m.get_axon_ntff_profile_hook = lambda: _h
        _m.set_axon_ntff_profile_hook = lambda h: None
        sys.modules["antenv.axon_hooks"] = _m
        antenv.axon_hooks = _m
except Exception:
    pass

import concourse.bass as bass
import concourse.bacc as bacc
import concourse.mybir as mybir
import concourse.tile as tile
from concourse import library_config
from concourse.bass_utils import run_bass_kernel_spmd

F32 = mybir.dt.float32
F16 = mybir.dt.float16
I16 = mybir.dt.int16
I32 = mybir.dt.int32
OP = mybir.AluOpType

N_CORES = 8
H = W = 2048
C = 8
RPC = H // N_CORES          # output rows per core = 256
PAD = 24                    # y halo rows on each side
YS = H + 2 * PAD            # 2096 slab rows
XS = 308                    # slab cols: [I0-24, I0+284)
XT = 176                    # table cols per row-group
XH = 88                     # x-half of the table staging buffer
YT = YS + 16                # table rows incl. pad so in_ap window stays in-bounds
SJW = 512                   # super-tile width (weights/idx granularity)
JW2 = 64                    # half-tile width (gather/combine granularity)
A = -0.75                   # bicubic constant
YB = 124                    # y-block rows for table build
N_YB = (YS + YB - 1) // YB  # 17


def build_nc():
    nc = bacc.Bacc("TRN2", target_bir_lowering=False, debug=False,
                   num_devices=N_CORES, num_swdge_queues=4)
    xs = nc.dram_tensor("xs", [C, YS + 4, XS], F16, kind="ExternalInput")
    gr = nc.dram_tensor("gr", [RPC, W, 2], F32, kind="ExternalInput")
    out = nc.dram_tensor("out", [RPC, W // JW2, C, JW2], F32,
                         kind="ExternalOutput")

    with tile.TileContext(nc) as tc:
        nc.gpsimd.load_library(library_config.mlp)
        import contextlib
        with contextlib.ExitStack() as ctx:
            _build_body(ctx, tc, nc, xs, gr, out)
    nc.compile()
    return nc


def _build_body(ctx, tc, nc, xs, gr, out):
    tabpool = ctx.enter_context(tc.tile_pool(name="tab", bufs=1, space="DRAM"))
    # phase-1 pools
    tpool = ctx.enter_context(tc.tile_pool(name="t", bufs=2))
    tg2pool = ctx.enter_context(tc.tile_pool(name="tg2", bufs=1))
    # phase-2 pools
    gridp = ctx.enter_context(tc.tile_pool(name="grid", bufs=2))
    wrk = ctx.enter_context(tc.tile_pool(name="wrk", bufs=1))
    wpp = ctx.enter_context(tc.tile_pool(name="wpp", bufs=2))
    idxp = ctx.enter_context(tc.tile_pool(name="idx", bufs=2))
    idxs1 = ctx.enter_context(tc.tile_pool(name="idx1", bufs=1))
    gp = ctx.enter_context(tc.tile_pool(name="g", bufs=2))
    lp = ctx.enter_context(tc.tile_pool(name="l", bufs=1))
    outp = ctx.enter_context(tc.tile_pool(name="out", bufs=4))

    tabs = []
    for g in range(2):
        tabg = tabpool.tile([YT * XT, 128], F16, tag=f"tab{g}")
        tabs.append(tabg)

    hwdge = [nc.sync, nc.scalar]
    cnt = {"dma": 0, "cp": 0, "q": 0}

    def eng():
        # phase-1 bulk traffic: alternate both HWDGE engines
        cnt["dma"] += 1
        return hwdge[cnt["dma"] % 2]

    def eng_prep():
        # phase-2 small DMAs go through SWDGE (gpsimd): the HWDGE queues are
        # backed up for ~1ms draining phase-1's table writes, and their
        # in-order FIFO would stall prep/output behind that drain.
        return nc.gpsimd

    def eng_out():
        return nc.gpsimd

    def ccopy(dst, src):
        cnt["cp"] += 1
        if cnt["cp"] % 2 == 0:
            nc.vector.tensor_copy(dst, src)
        else:
            nc.scalar.copy(dst, src)

    # ---------------- phase 1: repack xs -> table[g], one y-block ----------
    def build_block(g, yb):
        y0 = yb * YB
        rows = min(YB, YS - y0)
        # one DMA per row-shift r covering all 8 channels (slab is f16)
        t4 = tpool.tile([128, 4 * 8 * 179], F16, tag="xsb")
        for r in range(4):
            eng().dma_start(
                bass.AP(t4.tensor, t4.offset + r * 8 * 179,
                        [[t4.ap[0][0], rows], [1, 8 * 179]]),
                bass.AP(xs, (y0 + r) * XS + 128 * g,
                        [[XS, rows], [(YS + 4) * XS, 8], [1, 179]]))
        for h in range(2):
            tg2 = tg2pool.tile([128, XH * 128], F16, tag="tg2")
            for r in range(4):
                # merged interleave+shift:
                # tg2[p, xu*128 + s*32 + r*8 + c] = t[p, c*179 + XH*h + xu + s]
                dst = bass.AP(tg2.tensor, tg2.offset + r * 8,
                              [[tg2.ap[0][0], rows], [32, 4], [128, XH], [1, 8]])
                srcap = bass.AP(t4.tensor, t4.offset + r * 8 * 179 + XH * h,
                                [[t4.ap[0][0], rows], [1, 4], [1, XH], [179, 8]])
                ccopy(dst, srcap)
            # one HWDGE DMA, contiguous 22.5KB per row on both sides
            dsta = bass.AP(tabs[g].tensor,
                           tabs[g].offset + (y0 * XT + h * XH) * 128,
                           [[XT * 128, rows], [1, XH * 128]])
            eng().dma_start(dsta, tg2[:rows, :])

    # ---------------- phase 2: per super-tile weights+idx, gather+combine --
    def cubic(t, tag, outdt, opool):
        # returns w0..w3 tiles [128, SJW] in outdt; all-DVE (no cross-engine
        # handoffs); scratch tags shared between calls (sequential use).
        TS = nc.vector.tensor_scalar
        TT = nc.vector.tensor_tensor
        s0 = wrk.tile([128, SJW], F32, tag="c_s0")
        TS(s0[:], t[:], 1.0, None, op0=OP.add)
        w0f = wrk.tile([128, SJW], F32, tag="c_w0f")
        TS(w0f[:], s0[:], A, -5.0 * A, op0=OP.mult, op1=OP.add)
        TT(w0f[:], w0f[:], s0[:], op=OP.mult)
        TS(w0f[:], w0f[:], 8.0 * A, None, op0=OP.add)
        TT(w0f[:], w0f[:], s0[:], op=OP.mult)
        w0 = opool.tile([128, SJW], outdt, tag=f"w0{tag}")
        TS(w0[:], w0f[:], -4.0 * A, None, op0=OP.add)
        # w1
        w1f = wrk.tile([128, SJW], F32, tag="c_w1f")
        TS(w1f[:], t[:], A + 2.0, -(A + 3.0), op0=OP.mult, op1=OP.add)
        TT(w1f[:], w1f[:], t[:], op=OP.mult)
        TT(w1f[:], w1f[:], t[:], op=OP.mult)
        w1 = opool.tile([128, SJW], outdt, tag=f"w1{tag}")
        TS(w1[:], w1f[:], 1.0, None, op0=OP.add)
        # w2: u = 1 - t
        u = wrk.tile([128, SJW], F32, tag="c_u")
        TS(u[:], t[:], -1.0, 1.0, op0=OP.mult, op1=OP.add)
        w2f = wrk.tile([128, SJW], F32, tag="c_w2f")
        TS(w2f[:], u[:], A + 2.0, -(A + 3.0), op0=OP.mult, op1=OP.add)
        TT(w2f[:], w2f[:], u[:], op=OP.mult)
        TT(w2f[:], w2f[:], u[:], op=OP.mult)
        w2 = opool.tile([128, SJW], outdt, tag=f"w2{tag}")
        TS(w2[:], w2f[:], 1.0, None, op0=OP.add)
        # w3 = 1 - w0 - w1 - w2 (in f32 then cast)
        w3f = wrk.tile([128, SJW], F32, tag="c_w3f")
        TT(w3f[:], w0[:], w1[:], op=OP.add)
        TT(w3f[:], w3f[:], w2[:], op=OP.add)
        w3 = opool.tile([128, SJW], outdt, tag=f"w3{tag}")
        TS(w3[:], w3f[:], -1.0, 1.0, op0=OP.mult, op1=OP.add)
        return [w0, w1, w2, w3]

    def floorpair(v, tag):
        # vi/co scratch shared between calls; vf/fr persist per-dir
        vi = wrk.tile([128, SJW], I32, tag="f_vi")
        nc.vector.tensor_copy(vi[:], v[:])
        vf = wrk.tile([128, SJW], F32, tag=f"vf{tag}")
        nc.vector.tensor_copy(vf[:], vi[:])
        co = wrk.tile([128, SJW], F32, tag="f_co")
        nc.vector.tensor_tensor(co[:], vf[:], v[:], op=OP.is_gt)
        nc.vector.tensor_tensor(vf[:], vf[:], co[:], op=OP.subtract)
        fr = wrk.tile([128, SJW], F32, tag=f"fr{tag}")
        nc.vector.tensor_tensor(fr[:], v[:], vf[:], op=OP.subtract)
        return vf, fr

    def super_tile(g, s4):
        """Weights + wrapped idx for 512 output cols of row-group g.
        Returns (wp2, Cw): wp2[i, (s*4+r)*SJW + j] = wx_s(i,j)*wy_r(i,j)."""
        IG = g * 128
        jb4 = s4 * SJW
        gt = gridp.tile([128, SJW * 2], F32, tag="gt")
        nc.gpsimd.dma_start(
            gt[:],
            bass.AP(gr, IG * W * 2 + jb4 * 2, [[W * 2, 128], [1, SJW * 2]]))
        gx = bass.AP(gt.tensor, gt.offset, [gt.ap[0], [2, SJW]])
        gy = bass.AP(gt.tensor, gt.offset + 1, [gt.ap[0], [2, SJW]])

        lx = wrk.tile([128, SJW], F32, tag="lx")
        ly = wrk.tile([128, SJW], F32, tag="ly")
        nc.vector.tensor_scalar(lx[:], gx, 1024.0, 1047.5 - IG,
                                op0=OP.mult, op1=OP.add)
        nc.vector.tensor_scalar(ly[:], gy, 1024.0, 1046.5 - jb4,
                                op0=OP.mult, op1=OP.add)
        fx, tx = floorpair(lx, "x")
        fy, ty = floorpair(ly, "y")

        # idxf = fy*XT + fx - 1 (f32, exact)
        idxf = wrk.tile([128, SJW], F32, tag="idxf")
        nc.vector.scalar_tensor_tensor(idxf[:], fy[:], float(XT), fx[:],
                                       op0=OP.mult, op1=OP.add)
        # per-sub-tile rebase to the 186-row gather window, cast to i16
        idx16 = idxs1.tile([128, SJW], I16, tag="idx16")
        for t in range(SJW // 128):
            nc.vector.tensor_scalar(
                bass.AP(idx16.tensor, idx16.offset + t * 128,
                        [[idx16.ap[0][0], 128], [1, 128]]),
                bass.AP(idxf.tensor, idxf.offset + t * 128,
                        [[idxf.ap[0][0], 128], [1, 128]]),
                -1.0 - t * 128.0 * XT, None, op0=OP.add)

        # deferred steps (emitted interleaved with the previous supertile's
        # half-tiles so pool-queue waits resolve before issue):
        D = idxs1.tile([128, 8 * SJW], I16, tag="D")
        Cw = idxp.tile([128, 8 * SJW], I16, tag="Cw")

        def do_folds():
            # fold [128, SJW] -> wrapped [16, 8*SJW]: D[p, kS+j] = idx16[16k+p, j]
            for k in range(8):
                src = bass.AP(idx16.tensor,
                              idx16.offset + 16 * k * idx16.ap[0][0],
                              [[idx16.ap[0][0], 16], [1, SJW]])
                dst = bass.AP(D.tensor, D.offset + k * SJW,
                              [[D.ap[0][0], 16], [1, SJW]])
                eng_prep().dma_start(dst, src)

        def do_cwint():
            # interleave on the base band only (same DVE cycles as full
            # width); waits only on the fold DMAs, not the replication
            nc.vector.tensor_copy(
                bass.AP(Cw.tensor, Cw.offset, [[Cw.ap[0][0], 16], [8, SJW], [1, 8]]),
                bass.AP(D.tensor, D.offset, [[D.ap[0][0], 16], [1, SJW], [SJW, 8]]))

        def do_repl():
            # replicate Cw to all 8 gpsimd core bands: 7 independent DMAs
            # that drain under the remaining half-tiles (off the DVE path)
            for rep in range(1, 8):
                src = bass.AP(Cw.tensor, Cw.offset, [[Cw.ap[0][0], 16], [1, 8 * SJW]])
                dst = bass.AP(Cw.tensor, Cw.offset + 16 * rep * Cw.ap[0][0],
                              [[Cw.ap[0][0], 16], [1, 8 * SJW]])
                eng_prep().dma_start(dst, src)

        wx = cubic(tx, "x", F16, wrk)
        wy = cubic(ty, "y", F16, wrk)
        # wp2[i, (s*4+r)*SJW + j]: 16 fully-contiguous DVE multiplies
        wp2 = wpp.tile([128, 16 * SJW], F16, tag="wp2")
        for s in range(4):
            for r in range(4):
                dst = bass.AP(wp2.tensor, wp2.offset + (s * 4 + r) * SJW,
                              [wp2.ap[0], [1, SJW]])
                nc.vector.tensor_tensor(dst, wx[s][:], wy[r][:], op=OP.mult)
        return wp2, Cw, [do_folds, do_cwint, do_repl]

    def half_tile(g, s4, t, h, wp2, Cw):
        IG = g * 128
        jb = s4 * SJW + t * 128 + h * JW2
        ybase = s4 * SJW + t * 128
        joff = t * 128 + h * JW2

        # four 2048-idx gathers (ring-resident, 4-queue pipelined):
        # G[i, jl*128 + e], e = s*32 + r*8 + c
        G = gp.tile([128, JW2 * 128], F16, tag="G")
        in_ap = bass.AP(tabs[g].tensor,
                        tabs[g].offset + ybase * XT * 128,
                        [[128, 186 * XT], [1, 128]])
        NSUB = 2048
        for m in range(4):
            idxsl = bass.AP(Cw.tensor, Cw.offset + joff * 8 + m * (NSUB // 16),
                            [[Cw.ap[0][0], 128], [1, NSUB // 16]])
            q = cnt["q"] % 4
            cnt["q"] += 1
            nc.gpsimd.dma_gather(
                out_ap=bass.AP(G.tensor, G.offset + m * 16 * 128,
                               [[G.ap[0][0], 128], [128, 16], [1, 128]]),
                in_ap=in_ap,
                idxs_ap=idxsl,
                num_idxs=NSUB,
                num_idxs_reg=NSUB,
                elem_size=128,
                elem_step=128,
                single_packet=False,
                queue_num=q,
            )

        # combine: G[i, jl*128 + s*32 + r*8 + c] *= wp2[i, (s*4+r)*SJW + jb+jl]
        # (bcast over c) in two halves
        for m in range(2):
            src1 = bass.AP(wp2.tensor, wp2.offset + joff + m * 32,
                           [wp2.ap[0], [1, JW2 // 2], [4 * SJW, 4], [SJW, 4],
                            [0, 8]])
            src0 = bass.AP(G.tensor, G.offset + m * 32 * 128,
                           [G.ap[0], [128, JW2 // 2], [32, 4], [8, 4], [1, 8]])
            nc.vector.tensor_tensor(src0, src0, src1, op=OP.mult)

        def halve(buf, stride, n, tag, npx=JW2):
            o = lp.tile([128, npx * stride * (n // 2)], F16, tag=tag)
            i0 = bass.AP(buf.tensor, buf.offset,
                         [buf.ap[0], [stride * n, npx], [stride * 2, n // 2], [1, stride]])
            i1 = bass.AP(buf.tensor, buf.offset + stride,
                         [buf.ap[0], [stride * n, npx], [stride * 2, n // 2], [1, stride]])
            od = bass.AP(o.tensor, o.offset,
                         [o.ap[0], [stride * (n // 2), npx], [stride, n // 2], [1, stride]])
            nc.vector.tensor_tensor(od, i0, i1, op=OP.add)
            return o

        L1 = halve(G, 32, 4, "L1")
        L2 = halve(L1, 32, 2, "L2")
        L3 = halve(L2, 8, 4, "L3")
        of = outp.tile([128, 8 * JW2], F32, tag="of")
        i0 = bass.AP(L3.tensor, L3.offset, [L3.ap[0], [1, 8], [16, JW2]])
        i1 = bass.AP(L3.tensor, L3.offset + 8, [L3.ap[0], [1, 8], [16, JW2]])
        od = bass.AP(of.tensor, of.offset, [of.ap[0], [JW2, 8], [1, JW2]])
        nc.vector.tensor_tensor(od, i0, i1, op=OP.add)

        # blocked layout [RPC, W//64, C, 64]: 2KB contiguous per partition
        dsto = bass.AP(out, (IG * (W // JW2) + jb // JW2) * C * JW2,
                       [[(W // JW2) * C * JW2, 128], [1, C * JW2]])
        eng_out().dma_start(dsto, of[:])

    # ---------------- emission schedule --------------------------------
    # Emit the first two supertiles' prep BEFORE phase 1 so their weights/
    # idx/Cw DVE work runs ahead of phase-1's interleave-copy backlog in the
    # in-order DVE queue (their pool-side DMAs complete during the build).
    # g-major run order + tab[0]-first build lets g=0 gathers start as soon
    # as tab[0]'s writes drain while tab[1]'s drain underneath them.
    run_order = [(0, 0), (0, 1), (0, 2), (0, 3), (1, 0), (1, 1), (1, 2), (1, 3)]
    sups = {}
    for key in run_order[:2]:
        *head, steps = super_tile(*key)
        for st in steps:
            st()
        sups[key] = head

    for yb in range(N_YB):
        build_block(0, yb)
    for yb in range(N_YB):
        build_block(1, yb)

    for i, (g, s4) in enumerate(run_order):
        steps = []
        if i + 1 < len(run_order) and run_order[i + 1] not in sups:
            key = run_order[i + 1]
            *head, steps = super_tile(key[0], key[1])
            sups[key] = head
        # interleave the next supertile's idx-DMA chain between this one's
        # half-tiles: each link's producers finish well before the in-order
        # pool queue reaches it, so nothing blocks the gather stream
        slots = {2: 0, 4: 1, 5: 2}  # -> folds, cwint, repl
        k = 0
        for t in range(4):
            for h in range(2):
                half_tile(g, s4, t, h, *sups[(g, s4)])
                k += 1
                if k in slots and steps:
                    steps[slots[k]]()
        del sups[(g, s4)]


_NC_CACHE = None


def kernel(x: np.ndarray, grid: np.ndarray) -> np.ndarray:
    global _NC_CACHE
    if _NC_CACHE is None:
        _NC_CACHE = build_nc()
    nc = _NC_CACHE

    x0 = np.ascontiguousarray(x[0], dtype=np.float32)        # [C, H, W]
    g0 = np.ascontiguousarray(grid[0], dtype=np.float32)     # [H, W, 2]

    in_maps = []
    for k in range(N_CORES):
        I0 = k * RPC
        xsl = np.zeros((C, YS + 4, XS), dtype=np.float16)
        c0 = I0 - PAD
        lo, hi = max(0, c0), min(W, c0 + XS)
        xsl[:, PAD:PAD + H, lo - c0:hi - c0] = x0[:, :, lo:hi].astype(np.float16)
        grc = np.ascontiguousarray(g0[I0:I0 + RPC]).copy()
        grc[..., 0] -= I0 / 1024.0   # fold per-core x-base into gx
        in_maps.append({"xs": xsl, "gr": grc})

    res = run_bass_kernel_spmd(nc, in_maps, core_ids=list(range(N_CORES)),
                               trace=False)
    global _LAST_EXEC_NS, _LAST_RES
    _LAST_EXEC_NS = res.exec_time_ns
    _LAST_RES = res
    out = np.empty((1, C, H, W), dtype=np.float32)
    for k in range(N_CORES):
        blk = res.results[k]["out"]          # [RPC, W//64, C, 64]
        out[0, :, k * RPC:(k + 1) * RPC, :] = (
            blk.transpose(2, 0, 1, 3).reshape(C, RPC, W))
    return out


# revision 18
# speedup vs baseline: 1.3663x; 1.0381x over previous
"""Bicubic grid_sample (transpose-like warp) for Trainium2, 8 NeuronCores.

Strategy: shard output rows across cores (256 rows/core). The warp maps
output (i, j) -> input (y ~ j +- 21, x ~ i +- 21), so each core needs an
x-column slab of the image. On device, repack the slab into a patch table
in DRAM where each 256B unit holds the full 4x4x8ch bicubic patch at
(y0, x0) (fp16). v8: slab ships as f16 (halves slab DMA), weight
products built with fully-contiguous (s,r)-major DVE ops (no strided ACT
copies), phase-2 small DMAs issued via SWDGE (async rings; the HWDGE
FIFOs are busy draining phase-1's table writes), and the idx fold/
replicate/interleave chain for supertile k+1 is emitted between
supertile k's half-tiles so each link's producers finish before the
in-order pool queue reaches it (removes ~190us gather stalls at
supertile boundaries). Gathers stay 2048-idx x 4 queues: bigger calls
overflow the SWDGE descriptor rings and serialize desc-gen with drain.
Measured wall: the 256B-record random gather is SDMA per-descriptor
overhead-bound (~52ns fixed + ~0.1ns/B per engine); SBUF-source
transposed gather is worse (2B-per-partition xbar writes, ~30GB/s).
"""
import os, sys, types
sys.path.insert(0, "/opt/trn_rl_repo")
import numpy as np

try:  # register NTFF profile hook so BASS_TRACE=1 can measure HW time
    import antenv
    if "antenv.axon_hooks" not in sys.modules:
        from trn_agent_boot.trn_boot import _ntff_profile_via_ctypes
        _h = _ntff_profile_via_ctypes("/opt/axon/libaxon_pjrt.so")
        _m = types.ModuleType("antenv.axon_hooks")
        _m.get_axon_ntff_profile_hook = lambda: _h
        _m.set_axon_ntff_profile_hook = lambda h: None
        sys.modules["antenv.axon_hooks"] = _m
        antenv.axon_hooks = _m
except Exception:
    pass

import concourse.bass as bass
import concourse.bacc as bacc
import concourse.mybir as mybir
import concourse.tile as tile
from concourse import library_config
from concourse.bass_utils import run_bass_kernel_spmd

F32 = mybir.dt.float32
F16 = mybir.dt.float16
I16 = mybir.dt.int16
I32 = mybir.dt.int32
OP = mybir.AluOpType

N_CORES = 8
H = W = 2048
C = 8
RPC = H // N_CORES          # output rows per core = 256
PAD = 24                    # y halo rows on each side
YS = H + 2 * PAD            # 2096 slab rows
XS = 308                    # slab cols: [I0-24, I0+284)
XT = 176                    # table cols per row-group
XH = 44                     # x-quarter of the table staging buffer
YT = YS + 16                # table rows incl. pad so in_ap window stays in-bounds
SJW = 512                   # super-tile width (weights/idx granularity)
JW2 = 64                    # half-tile width (gather/combine granularity)
A = -0.75                   # bicubic constant
YB = 124                    # y-block rows for table build
N_YB = (YS + YB - 1) // YB  # 17


def build_nc():
    nc = bacc.Bacc("TRN2", target_bir_lowering=False, debug=False,
                   num_devices=N_CORES, num_swdge_queues=4)
    xs = nc.dram_tensor("xs", [C, YS + 4, XS], F16, kind="ExternalInput")
    gr = nc.dram_tensor("gr", [RPC, W, 2], F32, kind="ExternalInput")
    out = nc.dram_tensor("out", [RPC, W // JW2, C, JW2], F32,
                         kind="ExternalOutput")

    with tile.TileContext(nc) as tc:
        nc.gpsimd.load_library(library_config.mlp)
        import contextlib
        with contextlib.ExitStack() as ctx:
            _build_body(ctx, tc, nc, xs, gr, out)
    nc.compile()
    return nc


def _build_body(ctx, tc, nc, xs, gr, out):
    tabpool = ctx.enter_context(tc.tile_pool(name="tab", bufs=1, space="DRAM"))
    # phase-1 pools
    tpool = ctx.enter_context(tc.tile_pool(name="t", bufs=2))
    tg2pool = ctx.enter_context(tc.tile_pool(name="tg2", bufs=2))
    # phase-2 pools
    gridp = ctx.enter_context(tc.tile_pool(name="grid", bufs=2))
    wrk = ctx.enter_context(tc.tile_pool(name="wrk", bufs=1))
    wpp = ctx.enter_context(tc.tile_pool(name="wpp", bufs=2))
    idxp = ctx.enter_context(tc.tile_pool(name="idx", bufs=2))
    idxs1 = ctx.enter_context(tc.tile_pool(name="idx1", bufs=1))
    gp = ctx.enter_context(tc.tile_pool(name="g", bufs=2))
    lp = ctx.enter_context(tc.tile_pool(name="l", bufs=1))
    outp = ctx.enter_context(tc.tile_pool(name="out", bufs=4))

    tabs = []
    for g in range(2):
        tabg = tabpool.tile([YT * XT, 128], F16, tag=f"tab{g}")
        tabs.append(tabg)

    hwdge = [nc.sync, nc.scalar]
    cnt = {"dma": 0, "cp": 0, "q": 0}

    def eng():
        # phase-1 bulk traffic: alternate both HWDGE engines
        cnt["dma"] += 1
        return hwdge[cnt["dma"] % 2]

    def eng_prep():
        # phase-2 small DMAs go through SWDGE (gpsimd): the HWDGE queues are
        # backed up for ~1ms draining phase-1's table writes, and their
        # in-order FIFO would stall prep/output behind that drain.
        return nc.gpsimd

    def eng_out():
        return nc.gpsimd

    def ccopy(dst, src):
        cnt["cp"] += 1
        if cnt["cp"] % 2 == 0:
            nc.vector.tensor_copy(dst, src)
        else:
            nc.scalar.copy(dst, src)

    # ---------------- phase 1: repack xs -> table[g], one y-block ----------
    def build_block(g, yb):
        y0 = yb * YB
        rows = min(YB, YS - y0)
        # one DMA per row-shift r covering all 8 channels (slab is f16)
        t4 = tpool.tile([128, 4 * 8 * 179], F16, tag="xsb")
        for r in range(4):
            eng().dma_start(
                bass.AP(t4.tensor, t4.offset + r * 8 * 179,
                        [[t4.ap[0][0], rows], [1, 8 * 179]]),
                bass.AP(xs, (y0 + r) * XS + 128 * g,
                        [[XS, rows], [(YS + 4) * XS, 8], [1, 179]]))
        for h in range(4):
            tg2 = tg2pool.tile([128, XH * 128], F16, tag="tg2")
            for r in range(4):
                # merged interleave+shift:
                # tg2[p, xu*128 + s*32 + r*8 + c] = t[p, c*179 + XH*h + xu + s]
                dst = bass.AP(tg2.tensor, tg2.offset + r * 8,
                              [[tg2.ap[0][0], rows], [32, 4], [128, XH], [1, 8]])
                srcap = bass.AP(t4.tensor, t4.offset + r * 8 * 179 + XH * h,
                                [[t4.ap[0][0], rows], [1, 4], [1, XH], [179, 8]])
                ccopy(dst, srcap)
            # one HWDGE DMA, contiguous 11.3KB per row on both sides
            dsta = bass.AP(tabs[g].tensor,
                           tabs[g].offset + (y0 * XT + h * XH) * 128,
                           [[XT * 128, rows], [1, XH * 128]])
            eng().dma_start(dsta, tg2[:rows, :])

    # ---------------- phase 2: per super-tile weights+idx, gather+combine --
    def cubic(t, tag, outdt, opool):
        # returns w0..w3 tiles [128, SJW] in outdt; all-DVE (no cross-engine
        # handoffs); scratch tags shared between calls (sequential use).
        TS = nc.vector.tensor_scalar
        TT = nc.vector.tensor_tensor
        s0 = wrk.tile([128, SJW], F32, tag="c_s0")
        TS(s0[:], t[:], 1.0, None, op0=OP.add)
        w0f = wrk.tile([128, SJW], F32, tag="c_w0f")
        TS(w0f[:], s0[:], A, -5.0 * A, op0=OP.mult, op1=OP.add)
        TT(w0f[:], w0f[:], s0[:], op=OP.mult)
        TS(w0f[:], w0f[:], 8.0 * A, None, op0=OP.add)
        TT(w0f[:], w0f[:], s0[:], op=OP.mult)
        w0 = opool.tile([128, SJW], outdt, tag=f"w0{tag}")
        TS(w0[:], w0f[:], -4.0 * A, None, op0=OP.add)
        # w1
        w1f = wrk.tile([128, SJW], F32, tag="c_w1f")
        TS(w1f[:], t[:], A + 2.0, -(A + 3.0), op0=OP.mult, op1=OP.add)
        TT(w1f[:], w1f[:], t[:], op=OP.mult)
        TT(w1f[:], w1f[:], t[:], op=OP.mult)
        w1 = opool.tile([128, SJW], outdt, tag=f"w1{tag}")
        TS(w1[:], w1f[:], 1.0, None, op0=OP.add)
        # w2: u = 1 - t
        u = wrk.tile([128, SJW], F32, tag="c_u")
        TS(u[:], t[:], -1.0, 1.0, op0=OP.mult, op1=OP.add)
        w2f = wrk.tile([128, SJW], F32, tag="c_w2f")
        TS(w2f[:], u[:], A + 2.0, -(A + 3.0), op0=OP.mult, op1=OP.add)
        TT(w2f[:], w2f[:], u[:], op=OP.mult)
        TT(w2f[:], w2f[:], u[:], op=OP.mult)
        w2 = opool.tile([128, SJW], outdt, tag=f"w2{tag}")
        TS(w2[:], w2f[:], 1.0, None, op0=OP.add)
        # w3 = 1 - w0 - w1 - w2 (in f32 then cast)
        w3f = wrk.tile([128, SJW], F32, tag="c_w3f")
        TT(w3f[:], w0[:], w1[:], op=OP.add)
        TT(w3f[:], w3f[:], w2[:], op=OP.add)
        w3 = opool.tile([128, SJW], outdt, tag=f"w3{tag}")
        TS(w3[:], w3f[:], -1.0, 1.0, op0=OP.mult, op1=OP.add)
        return [w0, w1, w2, w3]

    def floorpair(v, tag):
        # vi/co scratch shared between calls; vf/fr persist per-dir
        vi = wrk.tile([128, SJW], I32, tag="f_vi")
        nc.vector.tensor_copy(vi[:], v[:])
        vf = wrk.tile([128, SJW], F32, tag=f"vf{tag}")
        nc.vector.tensor_copy(vf[:], vi[:])
        co = wrk.tile([128, SJW], F32, tag="f_co")
        nc.vector.tensor_tensor(co[:], vf[:], v[:], op=OP.is_gt)
        nc.vector.tensor_tensor(vf[:], vf[:], co[:], op=OP.subtract)
        fr = wrk.tile([128, SJW], F32, tag=f"fr{tag}")
        nc.vector.tensor_tensor(fr[:], v[:], vf[:], op=OP.subtract)
        return vf, fr

    def super_tile(g, s4):
        """Weights + wrapped idx for 512 output cols of row-group g.
        Returns (wp2, Cw): wp2[i, (s*4+r)*SJW + j] = wx_s(i,j)*wy_r(i,j)."""
        IG = g * 128
        jb4 = s4 * SJW
        gt = gridp.tile([128, SJW * 2], F32, tag="gt")
        nc.gpsimd.dma_start(
            gt[:],
            bass.AP(gr, IG * W * 2 + jb4 * 2, [[W * 2, 128], [1, SJW * 2]]))
        gx = bass.AP(gt.tensor, gt.offset, [gt.ap[0], [2, SJW]])
        gy = bass.AP(gt.tensor, gt.offset + 1, [gt.ap[0], [2, SJW]])

        lx = wrk.tile([128, SJW], F32, tag="lx")
        ly = wrk.tile([128, SJW], F32, tag="ly")
        nc.vector.tensor_scalar(lx[:], gx, 1024.0, 1047.5 - IG,
                                op0=OP.mult, op1=OP.add)
        nc.vector.tensor_scalar(ly[:], gy, 1024.0, 1046.5 - jb4,
                                op0=OP.mult, op1=OP.add)
        fx, tx = floorpair(lx, "x")
        fy, ty = floorpair(ly, "y")

        # idxf = fy*XT + fx - 1 (f32, exact)
        idxf = wrk.tile([128, SJW], F32, tag="idxf")
        nc.vector.scalar_tensor_tensor(idxf[:], fy[:], float(XT), fx[:],
                                       op0=OP.mult, op1=OP.add)
        # per-sub-tile rebase to the 186-row gather window, cast to i16
        idx16 = idxs1.tile([128, SJW], I16, tag="idx16")
        for t in range(SJW // 128):
            nc.vector.tensor_scalar(
                bass.AP(idx16.tensor, idx16.offset + t * 128,
                        [[idx16.ap[0][0], 128], [1, 128]]),
                bass.AP(idxf.tensor, idxf.offset + t * 128,
                        [[idxf.ap[0][0], 128], [1, 128]]),
                -1.0 - t * 128.0 * XT, None, op0=OP.add)

        # deferred steps (emitted interleaved with the previous supertile's
        # half-tiles so pool-queue waits resolve before issue):
        D = idxs1.tile([128, 8 * SJW], I16, tag="D")
        Cw = idxp.tile([128, 8 * SJW], I16, tag="Cw")

        def do_folds():
            # fold [128, SJW] -> wrapped [16, 8*SJW]: D[p, kS+j] = idx16[16k+p, j]
            for k in range(8):
                src = bass.AP(idx16.tensor,
                              idx16.offset + 16 * k * idx16.ap[0][0],
                              [[idx16.ap[0][0], 16], [1, SJW]])
                dst = bass.AP(D.tensor, D.offset + k * SJW,
                              [[D.ap[0][0], 16], [1, SJW]])
                eng_prep().dma_start(dst, src)

        def do_cwint():
            # interleave on the base band only (same DVE cycles as full
            # width); waits only on the fold DMAs, not the replication
            nc.vector.tensor_copy(
                bass.AP(Cw.tensor, Cw.offset, [[Cw.ap[0][0], 16], [8, SJW], [1, 8]]),
                bass.AP(D.tensor, D.offset, [[D.ap[0][0], 16], [1, SJW], [SJW, 8]]))

        def do_repl():
            # replicate Cw to all 8 gpsimd core bands: 7 independent DMAs
            # that drain under the remaining half-tiles (off the DVE path)
            for rep in range(1, 8):
                src = bass.AP(Cw.tensor, Cw.offset, [[Cw.ap[0][0], 16], [1, 8 * SJW]])
                dst = bass.AP(Cw.tensor, Cw.offset + 16 * rep * Cw.ap[0][0],
                              [[Cw.ap[0][0], 16], [1, 8 * SJW]])
                eng_prep().dma_start(dst, src)

        wx = cubic(tx, "x", F16, wrk)
        wy = cubic(ty, "y", F16, wrk)
        # wp2[i, (s*4+r)*SJW + j]: 16 fully-contiguous DVE multiplies
        wp2 = wpp.tile([128, 16 * SJW], F16, tag="wp2")
        for s in range(4):
            for r in range(4):
                dst = bass.AP(wp2.tensor, wp2.offset + (s * 4 + r) * SJW,
                              [wp2.ap[0], [1, SJW]])
                nc.vector.tensor_tensor(dst, wx[s][:], wy[r][:], op=OP.mult)
        return wp2, Cw, [do_folds, do_cwint, do_repl]

    def half_tile(g, s4, t, h, wp2, Cw):
        IG = g * 128
        jb = s4 * SJW + t * 128 + h * JW2
        ybase = s4 * SJW + t * 128
        joff = t * 128 + h * JW2

        # four 2048-idx gathers (ring-resident, 4-queue pipelined):
        # G[i, jl*128 + e], e = s*32 + r*8 + c
        G = gp.tile([128, JW2 * 128], F16, tag="G")
        in_ap = bass.AP(tabs[g].tensor,
                        tabs[g].offset + ybase * XT * 128,
                        [[128, 186 * XT], [1, 128]])
        NSUB = 2048
        for m in range(4):
            idxsl = bass.AP(Cw.tensor, Cw.offset + joff * 8 + m * (NSUB // 16),
                            [[Cw.ap[0][0], 128], [1, NSUB // 16]])
            q = cnt["q"] % 4
            cnt["q"] += 1
            nc.gpsimd.dma_gather(
                out_ap=bass.AP(G.tensor, G.offset + m * 16 * 128,
                               [[G.ap[0][0], 128], [128, 16], [1, 128]]),
                in_ap=in_ap,
                idxs_ap=idxsl,
                num_idxs=NSUB,
                num_idxs_reg=NSUB,
                elem_size=128,
                elem_step=128,
                single_packet=False,
                queue_num=q,
            )

        # combine: G[i, jl*128 + s*32 + r*8 + c] *= wp2[i, (s*4+r)*SJW + jb+jl]
        # (bcast over c) in two halves
        for m in range(2):
            src1 = bass.AP(wp2.tensor, wp2.offset + joff + m * 32,
                           [wp2.ap[0], [1, JW2 // 2], [4 * SJW, 4], [SJW, 4],
                            [0, 8]])
            src0 = bass.AP(G.tensor, G.offset + m * 32 * 128,
                           [G.ap[0], [128, JW2 // 2], [32, 4], [8, 4], [1, 8]])
            nc.vector.tensor_tensor(src0, src0, src1, op=OP.mult)

        def halve(buf, stride, n, tag, npx=JW2):
            o = lp.tile([128, npx * stride * (n // 2)], F16, tag=tag)
            i0 = bass.AP(buf.tensor, buf.offset,
                         [buf.ap[0], [stride * n, npx], [stride * 2, n // 2], [1, stride]])
            i1 = bass.AP(buf.tensor, buf.offset + stride,
                         [buf.ap[0], [stride * n, npx], [stride * 2, n // 2], [1, stride]])
            od = bass.AP(o.tensor, o.offset,
                         [o.ap[0], [stride * (n // 2), npx], [stride, n // 2], [1, stride]])
            nc.vector.tensor_tensor(od, i0, i1, op=OP.add)
            return o

        L1 = halve(G, 32, 4, "L1")
        L2 = halve(L1, 32, 2, "L2")
        L3 = halve(L2, 8, 4, "L3")
        of = outp.tile([128, 8 * JW2], F32, tag="of")
        i0 = bass.AP(L3.tensor, L3.offset, [L3.ap[0], [1, 8], [16, JW2]])
        i1 = bass.AP(L3.tensor, L3.offset + 8, [L3.ap[0], [1, 8], [16, JW2]])
        od = bass.AP(of.tensor, of.offset, [of.ap[0], [JW2, 8], [1, JW2]])
        nc.vector.tensor_tensor(od, i0, i1, op=OP.add)

        # blocked layout [RPC, W//64, C, 64]: 2KB contiguous per partition
        dsto = bass.AP(out, (IG * (W // JW2) + jb // JW2) * C * JW2,
                       [[(W // JW2) * C * JW2, 128], [1, C * JW2]])
        eng_out().dma_start(dsto, of[:])

    # ---------------- emission schedule --------------------------------
    # Emit the first two supertiles' prep BEFORE phase 1 so their weights/
    # idx/Cw DVE work runs ahead of phase-1's interleave-copy backlog in the
    # in-order DVE queue (their pool-side DMAs complete during the build).
    # g-major run order + tab[0]-first build lets g=0 gathers start as soon
    # as tab[0]'s writes drain while tab[1]'s drain underneath them.
    run_order = [(0, 0), (0, 1), (0, 2), (0, 3), (1, 0), (1, 1), (1, 2), (1, 3)]
    sups = {}
    for key in run_order[:2]:
        *head, steps = super_tile(*key)
        for st in steps:
            st()
        sups[key] = head

    for yb in range(N_YB):
        build_block(0, yb)
    for yb in range(N_YB):
        build_block(1, yb)

    for i, (g, s4) in enumerate(run_order):
        steps = []
        if i + 1 < len(run_order) and run_order[i + 1] not in sups:
            key = run_order[i + 1]
            *head, steps = super_tile(key[0], key[1])
            sups[key] = head
        # interleave the next supertile's idx-DMA chain between this one's
        # half-tiles: each link's producers finish well before the in-order
        # pool queue reaches it, so nothing blocks the gather stream
        slots = {2: 0, 4: 1, 5: 2}  # -> folds, cwint, repl
        k = 0
        for t in range(4):
            for h in range(2):
                half_tile(g, s4, t, h, *sups[(g, s4)])
                k += 1
                if k in slots and steps:
                    steps[slots[k]]()
        del sups[(g, s4)]


_NC_CACHE = None


def kernel(x: np.ndarray, grid: np.ndarray) -> np.ndarray:
    global _NC_CACHE
    if _NC_CACHE is None:
        _NC_CACHE = build_nc()
    nc = _NC_CACHE

    x0 = np.ascontiguousarray(x[0], dtype=np.float32)        # [C, H, W]
    g0 = np.ascontiguousarray(grid[0], dtype=np.float32)     # [H, W, 2]

    in_maps = []
    for k in range(N_CORES):
        I0 = k * RPC
        xsl = np.zeros((C, YS + 4, XS), dtype=np.float16)
        c0 = I0 - PAD
        lo, hi = max(0, c0), min(W, c0 + XS)
        xsl[:, PAD:PAD + H, lo - c0:hi - c0] = x0[:, :, lo:hi].astype(np.float16)
        grc = np.ascontiguousarray(g0[I0:I0 + RPC]).copy()
        grc[..., 0] -= I0 / 1024.0   # fold per-core x-base into gx
        in_maps.append({"xs": xsl, "gr": grc})

    res = run_bass_kernel_spmd(nc, in_maps, core_ids=list(range(N_CORES)),
                               trace=False)
    global _LAST_EXEC_NS, _LAST_RES
    _LAST_EXEC_NS = res.exec_time_ns
    _LAST_RES = res
    out = np.empty((1, C, H, W), dtype=np.float32)
    for k in range(N_CORES):
        blk = res.results[k]["out"]          # [RPC, W//64, C, 64]
        out[0, :, k * RPC:(k + 1) * RPC, :] = (
            blk.transpose(2, 0, 1, 3).reshape(C, RPC, W))
    return out
